# revision 1
# baseline (speedup 1.0000x reference)
"""Trainium2 Bass kernel: CrossAttention  (B=4, N=M=2048, D=1024, 16 heads x 64).

Sharding: 8 cores <- (batch, head-half): core c handles batch c//2, heads
(c%2)*8 .. (c%2)*8+8 (inner dims e = (c%2)*512 .. +512).  Each core computes
q/k/v projections for its slice, attention for its 8 heads, and the partial
output projection  yT_partial = WoT_loc.T @ outT_loc.  Host sums the two
partials per batch and adds the bias.

Device layout: transposed throughout (inner/contract dim on partitions):
  xT,cT [D, N], wqT/wkT/wvT [D, EL], woT [EL, D]; output yT [D, N].
All matmul operands are float32r (full PE rate at moving free-dim >= 256,
near-fp32 precision; measured end-to-end rel err ~1.4e-4).  Softmax is
max-free (logit scale ~0.4, safe for exp).  V is stored per head as
[v_h | ones] so one [65 x 512] matmul accumulates both the attention
output (rows 0-63) and the softmax denominator (row 64); normalization is
applied to the 64-row attention output via a K=33 broadcast matmul before
the output projection.  The QK sims row-pack two 64-dim heads into the
128x128 PE array (row tiling is the only fp32r-legal array packing; all
matmul outputs must start at psum partition 0).

Walrus/TRN2 constraint handled by _fix_pe_wait_overflow: every hardware
instruction has a single sync-wait slot, so the build post-processes the
scheduled module to merge/drop/relocate waits, helped by pinned NOP
wait-receivers emitted next to DMA bursts.
"""

import numpy as np
from contextlib import ExitStack

import concourse.bass as bass
import concourse.tile as tile
from concourse import mybir
from concourse.bass_utils import run_bass_kernel_spmd
from concourse.tile_rust import add_dep_helper

P = 128
FR = mybir.dt.float32r
F32 = mybir.dt.float32
EXP = mybir.ActivationFunctionType.Exp
MULT = mybir.AluOpType.mult

# problem dims (hardcoded per the harness contract)
B, NQ, NK, D = 4, 2048, 2048, 1024
HEADS, DIM_HEAD = 16, 64
INNER = HEADS * DIM_HEAD
EL = 512  # inner dims per core (8 heads)
SCALE = DIM_HEAD ** -0.5
IBS = 512  # query/key block size (psum bank free size)
N_CORES = 8


def build_module(D=D, NQ=NQ, NK=NK, EL=EL, trace_sim=False):
    n_ib = NQ // IBS   # query blocks
    n_jb = NK // IBS   # key blocks (projection granularity)
    n_jc = NK // P     # key chunks (attention contraction granularity)
    n_dc = D // P      # model-dim chunks
    n_pair = EL // P   # head pairs
    n_oc = D // P      # output-dim chunks

    nc = bass.Bass("TRN2", target_bir_lowering=False, debug=False)
    xT = nc.dram_tensor("xT", [D, NQ], FR, kind="ExternalInput").ap()
    cT = nc.dram_tensor("cT", [D, NK], FR, kind="ExternalInput").ap()
    wqT = nc.dram_tensor("wqT", [D, EL], FR, kind="ExternalInput").ap()
    wkT = nc.dram_tensor("wkT", [D, EL], FR, kind="ExternalInput").ap()
    wvT = nc.dram_tensor("wvT", [D, EL], FR, kind="ExternalInput").ap()
    woT = nc.dram_tensor("woT", [EL, D], FR, kind="ExternalInput").ap()
    yT = nc.dram_tensor("yT", [D, NQ], F32, kind="ExternalOutput").ap()

    xTr = xT.rearrange("(c p) n -> c p n", p=P)
    cTr = cT.rearrange("(c p) n -> c p n", p=P)
    wqTr = wqT.rearrange("(c p) e -> c p e", p=P)
    wkTr = wkT.rearrange("(c p) e -> c p e", p=P)
    wvTr = wvT.rearrange("(c p) e -> c p e", p=P)
    woTr = woT.rearrange("(c p) o -> c p o", p=P)
    yTr = yT.rearrange("(c p) n -> c p n", p=P)

    with tile.TileContext(nc, trace_sim=trace_sim) as tc, ExitStack() as ctx:

        constp = ctx.enter_context(tc.tile_pool(name="const", bufs=1))
        wop = ctx.enter_context(tc.tile_pool(name="wo", bufs=1))
        qtp = ctx.enter_context(tc.tile_pool(name="qt", bufs=1))
        ktp = ctx.enter_context(tc.tile_pool(name="kt", bufs=1))
        vp = ctx.enter_context(tc.tile_pool(name="v", bufs=1))
        wqp = ctx.enter_context(tc.tile_pool(name="wq", bufs=1))
        strp = ctx.enter_context(tc.tile_pool(name="stream", bufs=1))

        pj_ps = ctx.enter_context(tc.tile_pool(name="pjps", bufs=2, space="PSUM"))
        sim_ps = ctx.enter_context(tc.tile_pool(name="simps", bufs=1, space="PSUM"))
        av_ps = ctx.enter_context(tc.tile_pool(name="avps", bufs=2, space="PSUM"))

        # constants come in via DMA (walrus rejects memset on float32r):
        # col 0 = ones; cols 1..128 = sel (row 0 selects head-A output rows
        # 0-63, row 32 head-B rows 64-127); cols 132.. = zeros for rec init
        cst = nc.dram_tensor("cst", [P, 644], FR, kind="ExternalInput").ap()
        cstf = nc.dram_tensor("cstf", [33, P], F32, kind="ExternalInput").ap()
        cst_sb = constp.tile([P, 644], FR, name="cst", tag="cst")
        nc.sync.dma_start(cst_sb[:], cst[:])
        sel = constp.tile([33, P], F32, name="self32", tag="self32")
        nc.sync.dma_start(sel[:], cstf[:])
        # startup-only probe target: borrows the avA slot (released before
        # attention starts; slot reuse is same-engine and needs no sems)
        probe_ps = av_ps.tile([P, IBS], F32, name="prb", tag="avA")

        # Each PE matmul has a single HW wait slot, and fp32r matmuls are
        # self-loading (no separate ldweights to carry a second wait).
        # probe_src makes PE observe a freshly-DMA'd weight tile's queue
        # tick up front, so later matmuls reading (weights, activations)
        # carry only the activation-chunk queue tick.
        def probe_src(src):
            # K=1, dst [32, 64] at base 0 (a 1x1 dst fails walrus ISA checks)
            nc.tensor.matmul(probe_ps[0:32, 0:64], src[0:1, 0:32],
                             src[0:1, 0:64],
                             start=True, stop=True, skip_group_check=True)

        probe_src(cst_sb)

        # SP-stream dummies: dependency-free sequencer NOPs pinned after
        # `anchor` via a sync-free scheduling dep, so they sit between a
        # surplus wait's producer and the real DMA that needs to shed it —
        # free wait slots for _fix_pe_wait_overflow.
        def dummy_sp(anchor, n=1):
            for _ in range(n):
                h = nc.sync.nop()
                if anchor is not None:
                    add_dep_helper(h.ins, anchor.ins, sync=False,
                                   reason="pin SP dummy wait-receiver")

        def dummy_pe(anchor, n=1):
            for _ in range(n):
                h = nc.tensor.nop()
                if anchor is not None:
                    add_dep_helper(h.ins, anchor.ins, sync=False,
                                   reason="pin PE dummy wait-receiver")

        wo_sb = [wop.tile([P, D], FR, name=f"wo{c}", tag=f"wo{c}") for c in range(n_pair)]
        for c in range(n_pair):
            nc.sync.dma_start(wo_sb[c][:], woTr[c])
        for c in range(n_pair):
            probe_src(wo_sb[c])

        # qT is double-buffered by query block (half = ib%2) to save SBUF
        qT_sb = [qtp.tile([P, 2 * IBS], FR, name=f"q{c}", tag=f"q{c}") for c in range(n_pair)]
        kT_sb = [ktp.tile([P, NK], FR, name=f"k{c}", tag=f"k{c}") for c in range(n_pair)]
        # v is stored per head as [v_h | ones] (65 cols): the ones column
        # makes the AV matmul also produce the softmax denominator in row 64
        n_lh = 2 * n_pair
        v_sb = [vp.tile([P, n_lh * 65], FR, name=f"v{j}", tag=f"v{j}")
                for j in range(n_jc)]

        wq_sb = [wqp.tile([P, EL], FR, name=f"wq{c}", tag=f"wq{c}") for c in range(n_dc)]
        for c in range(n_dc):
            nc.sync.dma_start(wq_sb[c][:], wqTr[c])
        for c in range(n_dc):
            probe_src(wq_sb[c])

        # ---- k/v projections (whole NK) ----
        # note: no pool is ever closed — released-zone reuse would re-emit
        # extra waits on matmuls, which only have one HW wait slot.
        wkvp = ctx.enter_context(tc.tile_pool(name="wkv", bufs=1))
        expp = ctx.enter_context(tc.tile_pool(name="exp", bufs=3))
        outp = ctx.enter_context(tc.tile_pool(name="outT", bufs=2))
        recp = ctx.enter_context(tc.tile_pool(name="rec", bufs=1))
        ysbp = ctx.enter_context(tc.tile_pool(name="ysb", bufs=2))
        wk_sb = [wkvp.tile([P, EL], FR, name=f"wk{c}", tag=f"wk{c}") for c in range(n_dc)]
        wv_sb = [wkvp.tile([P, EL], FR, name=f"wv{c}", tag=f"wv{c}") for c in range(n_dc)]
        for c in range(n_dc):
            nc.sync.dma_start(wk_sb[c][:], wkTr[c])
            nc.sync.dma_start(wv_sb[c][:], wvTr[c])
        for c in range(n_dc):
            probe_src(wk_sb[c])
            probe_src(wv_sb[c])
        kv_anchor = None
        for jb in range(n_jb):
            if kv_anchor is not None:
                dummy_sp(kv_anchor, n=2 * n_dc)
            cs = [strp.tile([P, IBS], FR, name=f"s{c}", tag=f"s{c}") for c in range(n_dc)]
            for c in range(n_dc):
                kv_anchor = nc.sync.dma_start(
                    cs[c][:], cTr[c][:, jb * IBS:(jb + 1) * IBS])
            # kT[e, j] += wkT[d, e].T @ cT[d, j]
            for ec in range(n_pair):
                ps = pj_ps.tile([P, IBS], F32, name="pj", tag="pj")
                for dc in range(n_dc):
                    nc.tensor.matmul(
                        ps[:], wk_sb[dc][:, ec * P:(ec + 1) * P], cs[dc][:],
                        start=(dc == 0), stop=(dc == n_dc - 1))
                cp = nc.vector.tensor_copy(
                    kT_sb[ec][:, jb * IBS:(jb + 1) * IBS], ps[:])
                dummy_pe(cp, n=2)
            # v[j, e] += cT[d, j].T @ wvT[d, e]
            for jl in range(IBS // P):
                jc = jb * (IBS // P) + jl
                ps = pj_ps.tile([P, EL], F32, name="pj", tag="pj")
                for dc in range(n_dc):
                    nc.tensor.matmul(
                        ps[:], cs[dc][:, jl * P:(jl + 1) * P], wv_sb[dc][:],
                        start=(dc == 0), stop=(dc == n_dc - 1))
                for h in range(n_lh):
                    cp = nc.vector.tensor_copy(
                        v_sb[jc][:, h * 65:h * 65 + 64],
                        ps[:, h * 64:(h + 1) * 64])
                    nc.vector.tensor_copy(
                        v_sb[jc][:, h * 65 + 64:h * 65 + 65], cst_sb[:, 0:1])
                dummy_pe(cp, n=2)

        def qT_proj(ib, anchor=None):
            if anchor is not None:
                dummy_sp(anchor, n=2 * n_dc)
            xs = [strp.tile([P, IBS], FR, name=f"s{c}", tag=f"s{c}") for c in range(n_dc)]
            for c in range(n_dc):
                nc.sync.dma_start(xs[c][:], xTr[c][:, ib * IBS:(ib + 1) * IBS])
            for ec in range(n_pair):
                ps = pj_ps.tile([P, IBS], F32, name="pj", tag="pj")
                for dc in range(n_dc):
                    mm = nc.tensor.matmul(
                        ps[:], wq_sb[dc][:, ec * P:(ec + 1) * P], xs[dc][:],
                        start=(dc == 0), stop=(dc == n_dc - 1))
                    if ec == n_pair - 1 and dc == 0:
                        anchor_mm = mm
                cp = nc.vector.tensor_copy(
                    qT_sb[ec][:, (ib % 2) * IBS:(ib % 2 + 1) * IBS], ps[:])
                dummy_pe(cp, n=2)
            return anchor_mm

        # dummy ACT ops: scheduled before the first exp, they provide free
        # wait slots for _fix_pe_wait_overflow to relocate surplus waits to
        # (ACT has no earlier instructions to receive them otherwise).
        scratch = constp.tile([1, 8], F32, name="scratch", tag="scratch")
        scratch2 = constp.tile([1, 8], F32, name="scratch2", tag="scratch2")

        def dummy_dve(src_ap):
            # reads what DVE just wrote: unhoistable by the scheduler, and
            # the own-proc wait is dropped post-schedule -> free wait slot
            nc.vector.tensor_copy(scratch2[0:1, 0:1], src_ap)

        def dummy_act(src_ap):
            nc.scalar.copy(scratch[0:1, 0:1], src_ap)

        for _ in range(8):
            # read a late-arriving tile so the scheduler can't hoist these
            # to the very start (receivers must follow the waits' producers)
            nc.scalar.copy(scratch[0:1, 0:1], wq_sb[0][0:1, 0:1])

        q_anchor = qT_proj(0)

        for ib in range(n_ib):
            ib_sl = slice(ib * IBS, (ib + 1) * IBS)
            ot_tiles = []
            for pair in range(n_pair):
                avA = av_ps.tile([P, IBS], F32, name="avA", tag="avA")
                avB = av_ps.tile([P, IBS], F32, name="avB", tag="avB")
                hA, hB = 2 * pair, 2 * pair + 1
                # software-pipelined: sims+exps run one key-chunk ahead of
                # the AV accumulation so PE never idles on ACT latency
                q_sl = slice((ib % 2) * IBS, (ib % 2 + 1) * IBS)
                exps = [None] * n_jc

                def sim_exp(jc):
                    sA = sim_ps.tile([P, IBS], F32, name="sA", tag="sA")
                    sB = sim_ps.tile([P, IBS], F32, name="sB", tag="sB")
                    # simT[j, i] = kT[d, j].T @ qT[d, i], heads A/B row-packed
                    nc.tensor.matmul(
                        sA[:], kT_sb[pair][0:64, jc * P:(jc + 1) * P],
                        qT_sb[pair][0:64, q_sl],
                        start=True, stop=True, tile_position=(0, 0))
                    nc.tensor.matmul(
                        sB[:], kT_sb[pair][64:128, jc * P:(jc + 1) * P],
                        qT_sb[pair][64:128, q_sl],
                        start=True, stop=True, tile_position=(64, 0))
                    eA = expp.tile([P, IBS], FR, name="eA", tag="eA")
                    eB = expp.tile([P, IBS], FR, name="eB", tag="eB")
                    nc.scalar.activation(eA[:], sA[:], EXP, scale=SCALE)
                    nc.scalar.activation(eB[:], sB[:], EXP, scale=SCALE)
                    exps[jc] = (eA, eB)

                def av_acc(jc):
                    eA, eB = exps[jc]
                    st, sp = jc == 0, jc == n_jc - 1
                    # outT_unnorm[d, i] += [v_h | 1][j, :].T @ exp[j, i]:
                    # rows 0-63 = attention output, row 64 = denominator
                    nc.tensor.matmul(
                        avA[0:65, :], v_sb[jc][:, hA * 65:hA * 65 + 65], eA[:],
                        start=st, stop=sp, skip_group_check=True)
                    nc.tensor.matmul(
                        avB[0:65, :], v_sb[jc][:, hB * 65:hB * 65 + 65], eB[:],
                        start=st, stop=sp, skip_group_check=True)

                sim_exp(0)
                for jc in range(1, n_jc):
                    sim_exp(jc)
                    av_acc(jc - 1)
                av_acc(n_jc - 1)
                # normalize: outT[d, i] = outT_unnorm[d, i] / l[i] (per head)
                rec = recp.tile([33, IBS], F32, name="rec", tag="rec")
                nc.vector.memset(rec[:], 0.0)
                nc.vector.reciprocal(rec[0:1, :], avA[64:65, :])
                nc.vector.reciprocal(rec[32:33, :], avB[64:65, :])
                bc = pj_ps.tile([P, IBS], F32, name="bc", tag="pj")
                nc.tensor.matmul(bc[:], sel[0:33, :], rec[0:33, :],
                                 start=True, stop=True, skip_group_check=True)
                bcs = recp.tile([P, IBS], FR, name="bcs", tag="bcs")
                nc.vector.tensor_copy(bcs[:], bc[:])
                ot = outp.tile([P, IBS], FR, name=f"ot{pair}", tag=f"ot{pair}")
                nc.vector.tensor_tensor(ot[0:64, :], avA[0:64, :],
                                        bcs[0:64, :], MULT)
                nc.vector.tensor_tensor(ot[64:128, :], avB[0:64, :],
                                        bcs[64:128, :], MULT)
                ot_tiles.append(ot)
                dummy_dve(ot[0:1, 0:1])
                dummy_dve(ot[0:1, 1:2])
                dummy_act(exps[n_jc - 1][1][0:1, 0:1])
            if ib + 1 < n_ib:
                q_anchor = qT_proj(ib + 1, q_anchor)  # overlaps attention
            # yT_partial[o, i] += woT[e, o].T @ outT[e, i]
            for oc in range(n_oc):
                yp = pj_ps.tile([P, IBS], F32, name="pj", tag="pj")
                for pair in range(n_pair):
                    mm = nc.tensor.matmul(
                        yp[:], wo_sb[pair][:, oc * P:(oc + 1) * P],
                        ot_tiles[pair][:],
                        start=(pair == 0), stop=(pair == n_pair - 1))
                if oc == 0:
                    dummy_sp(mm, n=n_oc)
                ysb = ysbp.tile([P, IBS], F32, name="y", tag="y")
                ycp = nc.vector.tensor_copy(ysb[:], yp[:])
                dummy_sp(ycp, n=2)
                last_ydma = nc.sync.dma_start(yTr[oc][:, ib_sl], ysb[:])
                dummy_dve(ysb[0:1, 0:1])

        # tail receivers for the final barrier drain's 11 waits
        dummy_sp(last_ydma, n=12)

    _fix_pe_wait_overflow(nc)
    return nc


def _fix_pe_wait_overflow(nc):
    """Each hardware instruction has a single sync-wait slot (walrus: 'Too
    many sync wait commands').  Normalize every instruction to at most one
    wait in four phases:
      1. merge same-semaphore waits to the max value;
      2. drop own-proc waits (compute engines execute strictly in order);
      3. drop waits already implied by an earlier wait on the same engine
         (per-engine observed-tick tracking — Tile's emission is not
         transitive across instructions);
      4. move remaining surplus waits backwards onto an earlier same-engine
         instruction with a free slot (sem values are monotonic, so waiting
         earlier is strictly stronger; the scheduled block order is a
         topological order, so any receiver after the wait's producer
         cannot deadlock).
    """
    import bisect

    SKIP = ("InstISA", "InstEventSemaphore", "InstTriggerDma", "InstNoOp")
    ENG_SEM = {"EngineType.PE": "PE_", "EngineType.DVE": "DVE_",
               "EngineType.Activation": "Activation_",
               "EngineType.Pool": "Pool_", "EngineType.SP": "SP_"}

    # flatten all basic blocks (they execute sequentially) so cross-block
    # deps (e.g. the tail barrier drain) can relocate into earlier blocks
    if True:
        insts = []
        for blk in nc.m.functions[0].blocks:
            insts.extend(blk.instructions)
        cum = {}
        prod = {}  # sem name -> (cumulative values, instruction indices)
        for idx, i in enumerate(insts):
            si = i.sync_info
            if not si:
                continue
            for u in si.on_update:
                n = str(getattr(u, "ant_name", ""))
                v = cum.get(n, 0) + (u.update_value or 1)
                cum[n] = v
                vs, ids = prod.setdefault(n, ([], []))
                vs.append(v)
                ids.append(idx)

        def producer_idx(name, value):
            vs, ids = prod.get(name, ([], []))
            k = bisect.bisect_left(vs, value)
            return ids[k] if k < len(vs) else len(insts)

        def eng_of(i):
            return str(getattr(i, "engine", ""))

        def waits_of(i):
            si = i.sync_info
            return list(si.on_wait) if si else []

        def set_waits(i, ws):
            if i.sync_info is None:
                i.sync_info = mybir.SyncInfo(on_wait=ws, on_update=[])
            else:
                i.sync_info.on_wait = ws

        # phase 1+2: merge same-sem; drop own-proc waits
        for i in insts:
            if type(i).__name__ in SKIP:
                continue
            ws = waits_of(i)
            if not ws:
                continue
            best = {}
            for w in ws:
                n = str(getattr(w, "ant_name", ""))
                if n not in best or best[n].wait_value < w.wait_value:
                    best[n] = w
            own = ENG_SEM.get(eng_of(i))
            if own is not None and type(i).__name__ != "InstDMACopy":
                for n in list(best):
                    if n.startswith(own):
                        q = producer_idx(n, best[n].wait_value)
                        if q < len(insts) and eng_of(insts[q]) == eng_of(i) \
                                and type(insts[q]).__name__ not in SKIP:
                            del best[n]
            if len(best) != len(ws):
                set_waits(i, list(best.values()))

        def observed_sweep():
            # phase 3: per-engine observed ticks; drop implied waits
            obs = {}
            for i in insts:
                if type(i).__name__ in SKIP:
                    continue
                ws = waits_of(i)
                if not ws:
                    continue
                e = obs.setdefault(eng_of(i), {})
                kept = []
                for w in ws:
                    n = str(getattr(w, "ant_name", ""))
                    if e.get(n, -1) >= w.wait_value:
                        continue
                    kept.append(w)
                    e[n] = w.wait_value
                if len(kept) != len(ws):
                    set_waits(i, kept)

        observed_sweep()

        # phase 4: relocate surplus waits backwards (with cascading:
        # a receiver holding one wait can itself be freed by pushing its
        # wait further back, as long as every placement stays after the
        # corresponding producer)
        def can_receive(r_idx):
            t = type(insts[r_idx]).__name__
            return t == "InstNoOp" or t not in SKIP

        def place(w, lo, hi, eng, depth):
            """Place wait w on some same-engine instruction in (lo, hi).
            Returns True on success."""
            if depth <= 0:
                return False
            n = str(getattr(w, "ant_name", ""))
            for r in range(hi - 1, lo, -1):
                cand = insts[r]
                if eng_of(cand) != eng or not can_receive(r):
                    continue
                cw = waits_of(cand)
                if len(cw) == 0:
                    set_waits(cand, [w])
                    return True
                if len(cw) == 1:
                    cn = str(getattr(cw[0], "ant_name", ""))
                    if cn == n:
                        # same-sem: raising to max covers both
                        if cw[0].wait_value < w.wait_value:
                            set_waits(cand, [w])
                        return True
            # cascade: free a candidate by pushing its wait further back
            for r in range(hi - 1, lo, -1):
                cand = insts[r]
                if eng_of(cand) != eng or not can_receive(r):
                    continue
                cw = waits_of(cand)
                if len(cw) != 1 or type(cand).__name__ == "InstNoOp":
                    continue
                cq = producer_idx(str(getattr(cw[0], "ant_name", "")),
                                  cw[0].wait_value)
                if place(cw[0], cq, r, eng, depth - 1):
                    set_waits(cand, [w])
                    return True
            return False

        for idx, i in enumerate(insts):
            if type(i).__name__ in SKIP:
                continue
            ws = waits_of(i)
            if len(ws) <= 1:
                continue
            eng = eng_of(i)
            ws.sort(key=lambda w: producer_idx(
                str(getattr(w, "ant_name", "")), w.wait_value))
            remaining = list(ws)
            progress = True
            while len(remaining) > 1 and progress:
                progress = False
                for w in list(remaining):
                    if len(remaining) <= 1:
                        break
                    q = producer_idx(str(getattr(w, "ant_name", "")),
                                     w.wait_value)
                    if place(w, q, idx, eng, 4):
                        remaining.remove(w)
                        progress = True
                        break
            assert len(remaining) <= 1, (
                f"{i.name} ({eng}): cannot reduce waits "
                f"{[(str(w.ant_name), w.wait_value) for w in remaining]} "
                f"producers "
                f"{[producer_idx(str(w.ant_name), w.wait_value) for w in remaining]} "
                f"at idx {idx}")
            set_waits(i, remaining)

        observed_sweep()


_CACHE = {}


def _get_module():
    if "nc" not in _CACHE:
        _CACHE["nc"] = build_module()
    return _CACHE["nc"]


def make_in_maps(x, context, Wq, Wk, Wv, Wo):
    x = np.asarray(x, np.float32)
    context = np.asarray(context, np.float32)
    Wq = np.asarray(Wq, np.float32)
    Wk = np.asarray(Wk, np.float32)
    Wv = np.asarray(Wv, np.float32)
    Wo = np.asarray(Wo, np.float32)
    cst = np.zeros((P, 644), np.float32)
    cst[:, 0] = 1.0
    cstf = np.zeros((33, P), np.float32)
    cstf[0, 0:64] = 1.0
    cstf[32, 64:128] = 1.0
    in_maps = []
    for c in range(N_CORES):
        b, g = divmod(c, 2)
        sl = slice(g * EL, (g + 1) * EL)
        in_maps.append({
            "xT": np.ascontiguousarray(x[b].T),
            "cT": np.ascontiguousarray(context[b].T),
            "wqT": np.ascontiguousarray(Wq[sl].T),
            "wkT": np.ascontiguousarray(Wk[sl].T),
            "wvT": np.ascontiguousarray(Wv[sl].T),
            "woT": np.ascontiguousarray(Wo[:, sl].T),
            "cst": cst,
            "cstf": cstf,
        })
    return in_maps


def gather_output(results, bo):
    bo = np.asarray(bo, np.float32)
    y = np.empty((B, NQ, D), np.float32)
    for b in range(B):
        y[b] = (results[2 * b]["yT"] + results[2 * b + 1]["yT"]).T + bo
    return y


def kernel(x, context, Wq, Wk, Wv, Wo, bo):
    nc = _get_module()
    in_maps = make_in_maps(x, context, Wq, Wk, Wv, Wo)
    res = run_bass_kernel_spmd(nc, in_maps, core_ids=list(range(N_CORES)))
    return gather_output(res.results, bo)



# revision 8
# speedup vs baseline: 1.0931x; 1.0931x over previous
"""Trainium2 Bass kernel: CrossAttention  (B=4, N=M=2048, D=1024, 16 heads x 64).

Sharding: 8 cores <- (batch, head-half): core c handles batch c//2, heads
(c%2)*8 .. (c%2)*8+8 (inner dims e = (c%2)*512 .. +512).  Each core computes
q/k/v projections for its slice, attention for its 8 heads, and the partial
output projection  yT_partial = WoT_loc.T @ outT_loc.  Host sums the two
partials per batch and adds the bias.

Device layout: transposed throughout (inner/contract dim on partitions):
  xT,cT [D, N], wqT/wkT/wvT [D, EL], woT [EL, D]; output yT [D, N].
All matmul operands are float32r (full PE rate at moving free-dim >= 256,
near-fp32 precision; measured end-to-end rel err ~1.4e-4).  Softmax is
max-free (logit scale ~0.4, safe for exp).  V is stored per head as
[v_h | ones] so one [65 x 512] matmul accumulates both the attention
output (rows 0-63) and the softmax denominator (row 64); normalization is
applied to the 64-row attention output via a K=33 broadcast matmul before
the output projection.  The QK sims row-pack two 64-dim heads into the
128x128 PE array (row tiling is the only fp32r-legal array packing; all
matmul outputs must start at psum partition 0).

Walrus/TRN2 constraint handled by _fix_pe_wait_overflow: every hardware
instruction has a single sync-wait slot, so the build post-processes the
scheduled module to merge/drop/relocate waits, helped by pinned NOP
wait-receivers emitted next to DMA bursts.
"""

import numpy as np
from contextlib import ExitStack

import concourse.bass as bass
import concourse.tile as tile
from concourse import mybir
from concourse.bass_utils import run_bass_kernel_spmd
from concourse.tile_rust import add_dep_helper

P = 128
FR = mybir.dt.float32r
F32 = mybir.dt.float32
EXP = mybir.ActivationFunctionType.Exp
MULT = mybir.AluOpType.mult

# problem dims (hardcoded per the harness contract)
B, NQ, NK, D = 4, 2048, 2048, 1024
HEADS, DIM_HEAD = 16, 64
INNER = HEADS * DIM_HEAD
EL = 512  # inner dims per core (8 heads)
SCALE = DIM_HEAD ** -0.5
IBS = 512  # query/key block size (psum bank free size)
N_CORES = 8


def build_module(D=D, NQ=NQ, NK=NK, EL=EL, trace_sim=False):
    n_ib = NQ // IBS   # query blocks
    n_jb = NK // IBS   # key blocks (projection granularity)
    n_jc = NK // P     # key chunks (attention contraction granularity)
    n_dc = D // P      # model-dim chunks
    n_pair = EL // P   # head pairs
    n_oc = D // P      # output-dim chunks

    nc = bass.Bass("TRN2", target_bir_lowering=False, debug=False)
    xT = nc.dram_tensor("xT", [D, NQ], FR, kind="ExternalInput").ap()
    cT = nc.dram_tensor("cT", [D, NK], FR, kind="ExternalInput").ap()
    wqT = nc.dram_tensor("wqT", [D, EL], FR, kind="ExternalInput").ap()
    wkT = nc.dram_tensor("wkT", [D, EL], FR, kind="ExternalInput").ap()
    wvT = nc.dram_tensor("wvT", [D, EL], FR, kind="ExternalInput").ap()
    woT = nc.dram_tensor("woT", [EL, D], FR, kind="ExternalInput").ap()
    yT = nc.dram_tensor("yT", [D, NQ], F32, kind="ExternalOutput").ap()

    xTr = xT.rearrange("(c p) n -> c p n", p=P)
    cTr = cT.rearrange("(c p) n -> c p n", p=P)
    wqTr = wqT.rearrange("(c p) e -> c p e", p=P)
    wkTr = wkT.rearrange("(c p) e -> c p e", p=P)
    wvTr = wvT.rearrange("(c p) e -> c p e", p=P)
    woTr = woT.rearrange("(c p) o -> c p o", p=P)
    yTr = yT.rearrange("(c p) n -> c p n", p=P)

    with tile.TileContext(nc, trace_sim=trace_sim) as tc, ExitStack() as ctx:

        constp = ctx.enter_context(tc.tile_pool(name="const", bufs=1))
        wop = ctx.enter_context(tc.tile_pool(name="wo", bufs=1))
        qtp = ctx.enter_context(tc.tile_pool(name="qt", bufs=1))
        ktp = ctx.enter_context(tc.tile_pool(name="kt", bufs=1))
        vp = ctx.enter_context(tc.tile_pool(name="v", bufs=1))
        wqp = ctx.enter_context(tc.tile_pool(name="wq", bufs=1))
        strp = ctx.enter_context(tc.tile_pool(name="stream", bufs=1))

        pj_ps = ctx.enter_context(tc.tile_pool(name="pjps", bufs=2, space="PSUM"))
        sim_ps = ctx.enter_context(tc.tile_pool(name="simps", bufs=2, space="PSUM"))
        av_ps = ctx.enter_context(tc.tile_pool(name="avps", bufs=1, space="PSUM"))

        # constants come in via DMA (walrus rejects memset on float32r):
        # cols 0..7 = ones (v ones-fill + AV denominator rows); cols 8..135 =
        # sel rows... sel is its own FR tensor below.
        cst = nc.dram_tensor("cst", [P, 644], FR, kind="ExternalInput").ap()
        cstf = nc.dram_tensor("cstf", [33, P], FR, kind="ExternalInput").ap()
        cst_sb = constp.tile([P, 644], FR, name="cst", tag="cst")
        nc.sync.dma_start(cst_sb[:], cst[:])
        sel = constp.tile([33, P], FR, name="selfr", tag="selfr")
        nc.sync.dma_start(sel[:], cstf[:])
        # startup-only probe target: borrows the avA slot (released before
        # attention starts; slot reuse is same-engine and needs no sems)
        probe_ps = av_ps.tile([P, IBS], F32, name="prb", tag="avA")

        # Each PE matmul has a single HW wait slot, and fp32r matmuls are
        # self-loading (no separate ldweights to carry a second wait).
        # probe_src makes PE observe a freshly-DMA'd weight tile's queue
        # tick up front, so later matmuls reading (weights, activations)
        # carry only the activation-chunk queue tick.
        def probe_src(src):
            # K=1, dst [32, 64] at base 0 (a 1x1 dst fails walrus ISA checks)
            nc.tensor.matmul(probe_ps[0:32, 0:64], src[0:1, 0:32],
                             src[0:1, 0:64],
                             start=True, stop=True, skip_group_check=True)

        probe_src(cst_sb)

        # SP-stream dummies: dependency-free sequencer NOPs pinned after
        # `anchor` via a sync-free scheduling dep, so they sit between a
        # surplus wait's producer and the real DMA that needs to shed it —
        # free wait slots for _fix_pe_wait_overflow.
        def dummy_sp(anchor, n=1):
            for _ in range(n):
                h = nc.sync.nop()
                if anchor is not None:
                    add_dep_helper(h.ins, anchor.ins, sync=False,
                                   reason="pin SP dummy wait-receiver")

        def dummy_pe(anchor, n=1):
            for _ in range(n):
                h = nc.tensor.nop()
                if anchor is not None:
                    add_dep_helper(h.ins, anchor.ins, sync=False,
                                   reason="pin PE dummy wait-receiver")

        wo_sb = [wop.tile([P, D], FR, name=f"wo{c}", tag=f"wo{c}") for c in range(n_pair)]
        for c in range(n_pair):
            nc.sync.dma_start(wo_sb[c][:], woTr[c])
        for c in range(n_pair):
            probe_src(wo_sb[c])

        # qT is double-buffered by query block (half = ib%2) to save SBUF
        qT_sb = [qtp.tile([P, 2 * IBS], FR, name=f"q{c}", tag=f"q{c}") for c in range(n_pair)]
        kT_sb = [ktp.tile([P, NK], FR, name=f"k{c}", tag=f"k{c}") for c in range(n_pair)]
        # v is stored per head as [v_h | ones] (65 cols): the ones column
        # makes the AV matmul also produce the softmax denominator in row 64
        n_lh = 2 * n_pair
        v_sb = [vp.tile([P, n_lh * 65], FR, name=f"v{j}", tag=f"v{j}")
                for j in range(n_jc)]

        wq_sb = [wqp.tile([P, EL], FR, name=f"wq{c}", tag=f"wq{c}") for c in range(n_dc)]
        for c in range(n_dc):
            nc.sync.dma_start(wq_sb[c][:], wqTr[c])
        for c in range(n_dc):
            probe_src(wq_sb[c])

        # ---- k/v projections (whole NK) ----
        # note: no pool is ever closed — released-zone reuse would re-emit
        # extra waits on matmuls, which only have one HW wait slot.
        wkvp = ctx.enter_context(tc.tile_pool(name="wkv", bufs=1))
        expp = ctx.enter_context(tc.tile_pool(name="exp", bufs=3))
        outp = ctx.enter_context(tc.tile_pool(name="outT", bufs=2))
        recp = ctx.enter_context(tc.tile_pool(name="rec", bufs=1))
        avsbp = ctx.enter_context(tc.tile_pool(name="avsb", bufs=2))
        ysbp = ctx.enter_context(tc.tile_pool(name="ysb", bufs=2))
        wk_sb = [wkvp.tile([P, EL], FR, name=f"wk{c}", tag=f"wk{c}") for c in range(n_dc)]
        wv_sb = [wkvp.tile([P, EL], FR, name=f"wv{c}", tag=f"wv{c}") for c in range(n_dc)]
        for c in range(n_dc):
            nc.sync.dma_start(wk_sb[c][:], wkTr[c])
            nc.sync.dma_start(wv_sb[c][:], wvTr[c])
        for c in range(n_dc):
            probe_src(wk_sb[c])
            probe_src(wv_sb[c])
        kv_anchor = None
        for jb in range(n_jb):
            if kv_anchor is not None:
                dummy_sp(kv_anchor, n=2 * n_dc)
            cs = [strp.tile([P, IBS], FR, name=f"s{c}", tag=f"s{c}") for c in range(n_dc)]
            for c in range(n_dc):
                kv_anchor = nc.sync.dma_start(
                    cs[c][:], cTr[c][:, jb * IBS:(jb + 1) * IBS])
            # kT[e, j] += wkT[d, e].T @ cT[d, j]
            for ec in range(n_pair):
                ps = pj_ps.tile([P, IBS], F32, name="pj", tag="pj")
                for dc in range(n_dc):
                    nc.tensor.matmul(
                        ps[:], wk_sb[dc][:, ec * P:(ec + 1) * P], cs[dc][:],
                        start=(dc == 0), stop=(dc == n_dc - 1))
                cp = nc.vector.tensor_copy(
                    kT_sb[ec][:, jb * IBS:(jb + 1) * IBS], ps[:])
                dummy_pe(cp, n=2)
            # v[j, e] += cT[d, j].T @ wvT[d, e]
            for jl in range(IBS // P):
                jc = jb * (IBS // P) + jl
                ps = pj_ps.tile([P, EL], F32, name="pj", tag="pj")
                for dc in range(n_dc):
                    nc.tensor.matmul(
                        ps[:], cs[dc][:, jl * P:(jl + 1) * P], wv_sb[dc][:],
                        start=(dc == 0), stop=(dc == n_dc - 1))
                # one strided copy fills all 8 heads' v columns (dst stride
                # 65 skips the ones column), one more fills the ones columns
                dst = v_sb[jc][:, 0:n_lh * 65].rearrange(
                    "p (h c) -> p h c", c=65)
                src = ps[:, 0:n_lh * 64].rearrange("p (h c) -> p h c", c=64)
                cp = nc.vector.tensor_copy(dst[:, :, 0:64], src[:])
                nc.vector.tensor_copy(
                    dst[:, :, 64:65],
                    cst_sb[:, 0:n_lh].rearrange("p (h c) -> p h c", c=1))
                dummy_pe(cp, n=2)

        def qT_proj(ib, anchor=None):
            if anchor is not None:
                dummy_sp(anchor, n=2 * n_dc)
            xs = [strp.tile([P, IBS], FR, name=f"s{c}", tag=f"s{c}") for c in range(n_dc)]
            for c in range(n_dc):
                nc.sync.dma_start(xs[c][:], xTr[c][:, ib * IBS:(ib + 1) * IBS])
            for ec in range(n_pair):
                ps = pj_ps.tile([P, IBS], F32, name="pj", tag="pj")
                for dc in range(n_dc):
                    mm = nc.tensor.matmul(
                        ps[:], wq_sb[dc][:, ec * P:(ec + 1) * P], xs[dc][:],
                        start=(dc == 0), stop=(dc == n_dc - 1))
                    if ec == n_pair - 1 and dc == 0:
                        anchor_mm = mm
                cp = nc.vector.tensor_copy(
                    qT_sb[ec][:, (ib % 2) * IBS:(ib % 2 + 1) * IBS], ps[:])
                dummy_pe(cp, n=2)
            return anchor_mm

        # dummy ACT ops: scheduled before the first exp, they provide free
        # wait slots for _fix_pe_wait_overflow to relocate surplus waits to
        # (ACT has no earlier instructions to receive them otherwise).
        scratch = constp.tile([1, 8], F32, name="scratch", tag="scratch")
        scratch2 = constp.tile([1, 8], F32, name="scratch2", tag="scratch2")

        def dummy_dve(src_ap):
            # reads what DVE just wrote: unhoistable by the scheduler, and
            # the own-proc wait is dropped post-schedule -> free wait slot
            nc.vector.tensor_copy(scratch2[0:1, 0:1], src_ap)

        def dummy_act(src_ap):
            nc.scalar.copy(scratch[0:1, 0:1], src_ap)

        for _ in range(8):
            # read a late-arriving tile so the scheduler can't hoist these
            # to the very start (receivers must follow the waits' producers)
            nc.scalar.copy(scratch[0:1, 0:1], wq_sb[0][0:1, 0:1])

        # rec rows 1-31 must be zero for the sel broadcast matmul; rows 0/32
        # are rewritten per pair.  One startup fill (DMA — walrus rejects
        # memset on float32r) keeps them zero forever.
        rec = recp.tile([33, IBS], FR, name="rec", tag="rec")
        nc.sync.dma_start(rec[:], cst[0:33, 132:132 + IBS])

        q_anchor = qT_proj(0)

        for ib in range(n_ib):
            ib_sl = slice(ib * IBS, (ib + 1) * IBS)
            ot_tiles = []
            for pair in range(n_pair):
                avA = av_ps.tile([P, IBS], F32, name="avA", tag="avA")
                avB = av_ps.tile([P, IBS], F32, name="avB", tag="avB")
                hA, hB = 2 * pair, 2 * pair + 1
                # software-pipelined: sims+exps run one key-chunk ahead of
                # the AV accumulation so PE never idles on ACT latency
                q_sl = slice((ib % 2) * IBS, (ib % 2 + 1) * IBS)
                exps = [None] * n_jc

                def sim_exp(jc):
                    # both heads' sims land in one 2-bank psum tile so a
                    # single [128,1024] activation computes both exps
                    sAB = sim_ps.tile([P, 2 * IBS], F32, name="sAB", tag="sAB")
                    nc.tensor.matmul(
                        sAB[:, 0:IBS], kT_sb[pair][0:64, jc * P:(jc + 1) * P],
                        qT_sb[pair][0:64, q_sl],
                        start=True, stop=True, tile_position=(0, 0))
                    nc.tensor.matmul(
                        sAB[:, IBS:2 * IBS],
                        kT_sb[pair][64:128, jc * P:(jc + 1) * P],
                        qT_sb[pair][64:128, q_sl],
                        start=True, stop=True, tile_position=(64, 0))
                    eAB = expp.tile([P, 2 * IBS], FR, name="eAB", tag="eAB")
                    nc.scalar.activation(eAB[:], sAB[:], EXP, scale=SCALE)
                    exps[jc] = eAB

                def av_acc(jc):
                    eAB = exps[jc]
                    st, sp = jc == 0, jc == n_jc - 1
                    # outT_unnorm[d, i] += [v_h | 1][j, :].T @ exp[j, i]:
                    # rows 0-63 = attention output, row 64 = denominator
                    nc.tensor.matmul(
                        avA[0:65, :], v_sb[jc][:, hA * 65:hA * 65 + 65],
                        eAB[:, 0:IBS],
                        start=st, stop=sp, skip_group_check=True)
                    nc.tensor.matmul(
                        avB[0:65, :], v_sb[jc][:, hB * 65:hB * 65 + 65],
                        eAB[:, IBS:2 * IBS],
                        start=st, stop=sp, skip_group_check=True)

                sim_exp(0)
                for jc in range(1, n_jc):
                    sim_exp(jc)
                    av_acc(jc - 1)
                av_acc(n_jc - 1)
                # normalize: outT[d, i] = outT_unnorm[d, i] / l[i] (per head).
                # av is copied to SBUF first so its psum banks free up for the
                # next pair's accumulation (av pool is single-buffered).
                avsbA = avsbp.tile([65, IBS], F32, name="avsbA", tag="avsbA")
                avsbB = avsbp.tile([65, IBS], F32, name="avsbB", tag="avsbB")
                nc.vector.tensor_copy(avsbA[:], avA[0:65, :])
                nc.vector.tensor_copy(avsbB[:], avB[0:65, :])
                with nc.allow_low_precision(
                        reason="1/l stored as float32r for the fp32r "
                               "broadcast matmul; f32 bits either way"):
                    nc.vector.reciprocal(rec[0:1, :], avsbA[64:65, :])
                    nc.vector.reciprocal(rec[32:33, :], avsbB[64:65, :])
                bc = pj_ps.tile([P, IBS], F32, name="bc", tag="pj")
                nc.tensor.matmul(bc[:], sel[0:33, :], rec[0:33, :],
                                 start=True, stop=True, skip_group_check=True)
                ot = outp.tile([P, IBS], FR, name=f"ot{pair}", tag=f"ot{pair}")
                nc.vector.tensor_tensor(ot[0:64, :], avsbA[0:64, :],
                                        bc[0:64, :], MULT)
                nc.vector.tensor_tensor(ot[64:128, :], avsbB[0:64, :],
                                        bc[64:128, :], MULT)
                ot_tiles.append(ot)
                dummy_dve(ot[0:1, 0:1])
                dummy_dve(ot[0:1, 1:2])
                dummy_act(exps[n_jc - 1][0:1, 0:1])
            if ib + 1 < n_ib:
                q_anchor = qT_proj(ib + 1, q_anchor)  # overlaps attention
            # yT_partial[o, i] += woT[e, o].T @ outT[e, i]
            for oc in range(n_oc):
                yp = pj_ps.tile([P, IBS], F32, name="pj", tag="pj")
                for pair in range(n_pair):
                    mm = nc.tensor.matmul(
                        yp[:], wo_sb[pair][:, oc * P:(oc + 1) * P],
                        ot_tiles[pair][:],
                        start=(pair == 0), stop=(pair == n_pair - 1))
                if oc == 0:
                    dummy_sp(mm, n=n_oc)
                ysb = ysbp.tile([P, IBS], F32, name="y", tag="y")
                ycp = nc.vector.tensor_copy(ysb[:], yp[:])
                dummy_sp(ycp, n=2)
                last_ydma = nc.sync.dma_start(yTr[oc][:, ib_sl], ysb[:])
                dummy_dve(ysb[0:1, 0:1])

        # tail receivers for the final barrier drain's 11 waits
        dummy_sp(last_ydma, n=12)

    _fix_pe_wait_overflow(nc)
    return nc


def _fix_pe_wait_overflow(nc):
    """Each hardware instruction has a single sync-wait slot (walrus: 'Too
    many sync wait commands').  Normalize every instruction to at most one
    wait in four phases:
      1. merge same-semaphore waits to the max value;
      2. drop own-proc waits (compute engines execute strictly in order);
      3. drop waits already implied by an earlier wait on the same engine
         (per-engine observed-tick tracking — Tile's emission is not
         transitive across instructions);
      4. move remaining surplus waits backwards onto an earlier same-engine
         instruction with a free slot (sem values are monotonic, so waiting
         earlier is strictly stronger; the scheduled block order is a
         topological order, so any receiver after the wait's producer
         cannot deadlock).
    """
    import bisect

    SKIP = ("InstISA", "InstEventSemaphore", "InstTriggerDma", "InstNoOp")
    ENG_SEM = {"EngineType.PE": "PE_", "EngineType.DVE": "DVE_",
               "EngineType.Activation": "Activation_",
               "EngineType.Pool": "Pool_", "EngineType.SP": "SP_"}

    # flatten all basic blocks (they execute sequentially) so cross-block
    # deps (e.g. the tail barrier drain) can relocate into earlier blocks
    if True:
        insts = []
        for blk in nc.m.functions[0].blocks:
            insts.extend(blk.instructions)
        cum = {}
        prod = {}  # sem name -> (cumulative values, instruction indices)
        for idx, i in enumerate(insts):
            si = i.sync_info
            if not si:
                continue
            for u in si.on_update:
                n = str(getattr(u, "ant_name", ""))
                v = cum.get(n, 0) + (u.update_value or 1)
                cum[n] = v
                vs, ids = prod.setdefault(n, ([], []))
                vs.append(v)
                ids.append(idx)

        def producer_idx(name, value):
            vs, ids = prod.get(name, ([], []))
            k = bisect.bisect_left(vs, value)
            return ids[k] if k < len(vs) else len(insts)

        def eng_of(i):
            return str(getattr(i, "engine", ""))

        def waits_of(i):
            si = i.sync_info
            return list(si.on_wait) if si else []

        def set_waits(i, ws):
            if i.sync_info is None:
                i.sync_info = mybir.SyncInfo(on_wait=ws, on_update=[])
            else:
                i.sync_info.on_wait = ws

        # phase 1+2: merge same-sem; drop own-proc waits
        for i in insts:
            if type(i).__name__ in SKIP:
                continue
            ws = waits_of(i)
            if not ws:
                continue
            best = {}
            for w in ws:
                n = str(getattr(w, "ant_name", ""))
                if n not in best or best[n].wait_value < w.wait_value:
                    best[n] = w
            own = ENG_SEM.get(eng_of(i))
            if own is not None and type(i).__name__ != "InstDMACopy":
                for n in list(best):
                    if n.startswith(own):
                        q = producer_idx(n, best[n].wait_value)
                        if q < len(insts) and eng_of(insts[q]) == eng_of(i) \
                                and type(insts[q]).__name__ not in SKIP:
                            del best[n]
            if len(best) != len(ws):
                set_waits(i, list(best.values()))

        def observed_sweep():
            # phase 3: per-engine observed ticks; drop implied waits
            obs = {}
            for i in insts:
                if type(i).__name__ in SKIP:
                    continue
                ws = waits_of(i)
                if not ws:
                    continue
                e = obs.setdefault(eng_of(i), {})
                kept = []
                for w in ws:
                    n = str(getattr(w, "ant_name", ""))
                    if e.get(n, -1) >= w.wait_value:
                        continue
                    kept.append(w)
                    e[n] = w.wait_value
                if len(kept) != len(ws):
                    set_waits(i, kept)

        observed_sweep()

        # phase 4: relocate surplus waits backwards (with cascading:
        # a receiver holding one wait can itself be freed by pushing its
        # wait further back, as long as every placement stays after the
        # corresponding producer)
        def can_receive(r_idx):
            t = type(insts[r_idx]).__name__
            return t == "InstNoOp" or t not in SKIP

        def place(w, lo, hi, eng, depth):
            """Place wait w on some same-engine instruction in (lo, hi).
            Returns True on success."""
            if depth <= 0:
                return False
            n = str(getattr(w, "ant_name", ""))
            for r in range(hi - 1, lo, -1):
                cand = insts[r]
                if eng_of(cand) != eng or not can_receive(r):
                    continue
                cw = waits_of(cand)
                if len(cw) == 0:
                    set_waits(cand, [w])
                    return True
                if len(cw) == 1:
                    cn = str(getattr(cw[0], "ant_name", ""))
                    if cn == n:
                        # same-sem: raising to max covers both
                        if cw[0].wait_value < w.wait_value:
                            set_waits(cand, [w])
                        return True
            # cascade: free a candidate by pushing its wait further back
            for r in range(hi - 1, lo, -1):
                cand = insts[r]
                if eng_of(cand) != eng or not can_receive(r):
                    continue
                cw = waits_of(cand)
                if len(cw) != 1 or type(cand).__name__ == "InstNoOp":
                    continue
                cq = producer_idx(str(getattr(cw[0], "ant_name", "")),
                                  cw[0].wait_value)
                if place(cw[0], cq, r, eng, depth - 1):
                    set_waits(cand, [w])
                    return True
            return False

        for idx, i in enumerate(insts):
            if type(i).__name__ in SKIP:
                continue
            ws = waits_of(i)
            if len(ws) <= 1:
                continue
            eng = eng_of(i)
            ws.sort(key=lambda w: producer_idx(
                str(getattr(w, "ant_name", "")), w.wait_value))
            remaining = list(ws)
            progress = True
            while len(remaining) > 1 and progress:
                progress = False
                for w in list(remaining):
                    if len(remaining) <= 1:
                        break
                    q = producer_idx(str(getattr(w, "ant_name", "")),
                                     w.wait_value)
                    if place(w, q, idx, eng, 4):
                        remaining.remove(w)
                        progress = True
                        break
            assert len(remaining) <= 1, (
                f"{i.name} ({eng}): cannot reduce waits "
                f"{[(str(w.ant_name), w.wait_value) for w in remaining]} "
                f"producers "
                f"{[producer_idx(str(w.ant_name), w.wait_value) for w in remaining]} "
                f"at idx {idx}")
            set_waits(i, remaining)

        observed_sweep()


_CACHE = {}


def _get_module():
    if "nc" not in _CACHE:
        _CACHE["nc"] = build_module()
    return _CACHE["nc"]


def make_in_maps(x, context, Wq, Wk, Wv, Wo):
    x = np.asarray(x, np.float32)
    context = np.asarray(context, np.float32)
    Wq = np.asarray(Wq, np.float32)
    Wk = np.asarray(Wk, np.float32)
    Wv = np.asarray(Wv, np.float32)
    Wo = np.asarray(Wo, np.float32)
    cst = np.zeros((P, 644), np.float32)
    cst[:, 0:16] = 1.0
    cstf = np.zeros((33, P), np.float32)
    cstf[0, 0:64] = 1.0
    cstf[32, 64:128] = 1.0
    in_maps = []
    for c in range(N_CORES):
        b, g = divmod(c, 2)
        sl = slice(g * EL, (g + 1) * EL)
        in_maps.append({
            "xT": np.ascontiguousarray(x[b].T),
            "cT": np.ascontiguousarray(context[b].T),
            "wqT": np.ascontiguousarray(Wq[sl].T),
            "wkT": np.ascontiguousarray(Wk[sl].T),
            "wvT": np.ascontiguousarray(Wv[sl].T),
            "woT": np.ascontiguousarray(Wo[:, sl].T),
            "cst": cst,
            "cstf": cstf,
        })
    return in_maps


def gather_output(results, bo):
    bo = np.asarray(bo, np.float32)
    y = np.empty((B, NQ, D), np.float32)
    for b in range(B):
        y[b] = (results[2 * b]["yT"] + results[2 * b + 1]["yT"]).T + bo
    return y


def kernel(x, context, Wq, Wk, Wv, Wo, bo):
    nc = _get_module()
    in_maps = make_in_maps(x, context, Wq, Wk, Wv, Wo)
    res = run_bass_kernel_spmd(nc, in_maps, core_ids=list(range(N_CORES)))
    return gather_output(res.results, bo)



# revision 16
# speedup vs baseline: 1.2599x; 1.1526x over previous
"""Trainium2 Bass kernel: CrossAttention  (B=4, N=M=2048, D=1024, 16 heads x 64).

Sharding: 8 cores <- (batch, head-half): core c handles batch c//2, heads
(c%2)*8 .. (c%2)*8+8 (inner dims e = (c%2)*512 .. +512).  Each core computes
q/k/v projections for its slice, attention for its 8 heads, and the partial
output projection  yT_partial = WoT_loc.T @ outT_loc.  Host sums the two
partials per batch and adds the bias.

Device layout: transposed throughout (inner/contract dim on partitions):
  xT,cT [D, N], wqT/wkT/wvT [D, EL], woT [EL, D]; output yT [D, N].
All matmul operands are float32r (full PE rate at moving free-dim >= 256,
near-fp32 precision; measured end-to-end rel err ~1.4e-4).  Softmax is
max-free (logit scale ~0.4, safe for exp).  V is stored per head as
[v_h | ones] so one [65 x 512] matmul accumulates both the attention
output (rows 0-63) and the softmax denominator (row 64); normalization is
applied to the 64-row attention output via a K=33 broadcast matmul before
the output projection.  The QK sims row-pack two 64-dim heads into the
128x128 PE array (row tiling is the only fp32r-legal array packing; all
matmul outputs must start at psum partition 0).

Walrus/TRN2 constraint handled by _fix_pe_wait_overflow: every hardware
instruction has a single sync-wait slot, so the build post-processes the
scheduled module to merge/drop/relocate waits, helped by pinned NOP
wait-receivers emitted next to DMA bursts.
"""

import numpy as np
from contextlib import ExitStack

import concourse.bass as bass
import concourse.tile as tile
from concourse import mybir
from concourse.bass_utils import run_bass_kernel_spmd
from concourse.tile_rust import add_dep_helper

P = 128
FR = mybir.dt.float32r
F32 = mybir.dt.float32
BF16 = mybir.dt.bfloat16
EXP = mybir.ActivationFunctionType.Exp
MULT = mybir.AluOpType.mult

# problem dims (hardcoded per the harness contract)
B, NQ, NK, D = 4, 2048, 2048, 1024
HEADS, DIM_HEAD = 16, 64
INNER = HEADS * DIM_HEAD
EL = 512  # inner dims per core (8 heads)
SCALE = DIM_HEAD ** -0.5
IBS = 512  # query/key block size (psum bank free size)
N_CORES = 8


def build_module(D=D, NQ=NQ, NK=NK, EL=EL, trace_sim=False):
    n_ib = NQ // IBS   # query blocks
    n_jb = NK // IBS   # key blocks (projection granularity)
    n_jc = NK // P     # key chunks (attention contraction granularity)
    n_dc = D // P      # model-dim chunks
    n_pair = EL // P   # head pairs
    n_oc = D // P      # output-dim chunks

    nc = bass.Bass("TRN2", target_bir_lowering=False, debug=False)
    # projection operands (x/c streams + q/k/v weights) are bf16: the PE rate
    # is identical, DMA bytes and SBUF halve, and the precision cost (~0.3%
    # per projection) is far inside the 2e-2 budget.
    xT = nc.dram_tensor("xT", [D, NQ], BF16, kind="ExternalInput").ap()
    cT = nc.dram_tensor("cT", [D, NK], BF16, kind="ExternalInput").ap()
    wqT = nc.dram_tensor("wqT", [D, EL], BF16, kind="ExternalInput").ap()
    wkT = nc.dram_tensor("wkT", [D, EL], BF16, kind="ExternalInput").ap()
    wvT = nc.dram_tensor("wvT", [D, EL], BF16, kind="ExternalInput").ap()
    woT = nc.dram_tensor("woT", [EL, D], FR, kind="ExternalInput").ap()
    yT = nc.dram_tensor("yT", [D, NQ], F32, kind="ExternalOutput").ap()

    xTr = xT.rearrange("(c p) n -> c p n", p=P)
    cTr = cT.rearrange("(c p) n -> c p n", p=P)
    wqTr = wqT.rearrange("(c p) e -> c p e", p=P)
    wkTr = wkT.rearrange("(c p) e -> c p e", p=P)
    wvTr = wvT.rearrange("(c p) e -> c p e", p=P)
    woTr = woT.rearrange("(c p) o -> c p o", p=P)
    yTr = yT.rearrange("(c p) n -> c p n", p=P)

    with tile.TileContext(nc, trace_sim=trace_sim) as tc, ExitStack() as ctx:

        constp = ctx.enter_context(tc.tile_pool(name="const", bufs=1))
        wop = ctx.enter_context(tc.tile_pool(name="wo", bufs=1))
        qtp = ctx.enter_context(tc.tile_pool(name="qt", bufs=1))
        ktp = ctx.enter_context(tc.tile_pool(name="kt", bufs=1))
        vp = ctx.enter_context(tc.tile_pool(name="v", bufs=1))
        wqp = ctx.enter_context(tc.tile_pool(name="wq", bufs=1))
        # double-buffered so the next block's x/c DMAs overlap this block's
        # projection matmuls
        strp = ctx.enter_context(tc.tile_pool(name="stream", bufs=2))

        pj_ps = ctx.enter_context(tc.tile_pool(name="pjps", bufs=2, space="PSUM"))
        sim_ps = ctx.enter_context(tc.tile_pool(name="simps", bufs=2, space="PSUM"))
        av_ps = ctx.enter_context(tc.tile_pool(name="avps", bufs=1, space="PSUM"))

        # constants come in via DMA (walrus rejects memset on float32r):
        # cols 0..7 = ones (v ones-fill + AV denominator rows); cols 8..135 =
        # sel rows... sel is its own FR tensor below.
        cst = nc.dram_tensor("cst", [P, 644], FR, kind="ExternalInput").ap()
        cstf = nc.dram_tensor("cstf", [33, P], FR, kind="ExternalInput").ap()
        cst_sb = constp.tile([P, 644], FR, name="cst", tag="cst")
        sel = constp.tile([33, P], FR, name="selfr", tag="selfr")
        # startup-only probe target: borrows the avA slot (released before
        # attention starts; slot reuse is same-engine and needs no sems)
        probe_ps = av_ps.tile([P, IBS], F32, name="prb", tag="avA")

        # Each PE matmul has a single HW wait slot, and fp32r matmuls are
        # self-loading (no separate ldweights to carry a second wait).
        # probe_src makes PE observe a freshly-DMA'd weight tile's queue
        # tick up front, so later matmuls reading (weights, activations)
        # carry only the activation-chunk queue tick.
        def probe_src(src):
            # K=1, dst [32, 64] at base 0 (a 1x1 dst fails walrus ISA checks)
            nc.tensor.matmul(probe_ps[0:32, 0:64], src[0:1, 0:32],
                             src[0:1, 0:64],
                             start=True, stop=True, skip_group_check=True)

        probe_src(cst_sb)

        # SP-stream dummies: dependency-free sequencer NOPs pinned after
        # `anchor` via a sync-free scheduling dep, so they sit between a
        # surplus wait's producer and the real DMA that needs to shed it —
        # free wait slots for _fix_pe_wait_overflow.
        def dummy_sp(anchor, n=1):
            for _ in range(n):
                h = nc.sync.nop()
                if anchor is not None:
                    add_dep_helper(h.ins, anchor.ins, sync=False,
                                   reason="pin SP dummy wait-receiver")

        def dummy_pe(anchor, n=1):
            for _ in range(n):
                h = nc.tensor.nop()
                if anchor is not None:
                    add_dep_helper(h.ins, anchor.ins, sync=False,
                                   reason="pin PE dummy wait-receiver")

        wo_sb = [wop.tile([P, D], FR, name=f"wo{c}", tag=f"wo{c}") for c in range(n_pair)]
        for c in range(n_pair):
            nc.sync.dma_start(wo_sb[c][:], woTr[c])
        for c in range(n_pair):
            probe_src(wo_sb[c])

        # qT is double-buffered by query block (half = ib%2) to save SBUF
        qT_sb = [qtp.tile([P, 2 * IBS], FR, name=f"q{c}", tag=f"q{c}") for c in range(n_pair)]
        kT_sb = [ktp.tile([P, NK], FR, name=f"k{c}", tag=f"k{c}") for c in range(n_pair)]
        # v is stored per head as [v_h | ones] (65 cols): the ones column
        # makes the AV matmul also produce the softmax denominator in row 64
        n_lh = 2 * n_pair
        v_sb = [vp.tile([P, n_lh * 65], FR, name=f"v{j}", tag=f"v{j}")
                for j in range(n_jc)]

        wq_sb = [wqp.tile([P, EL], BF16, name=f"wq{c}", tag=f"wq{c}") for c in range(n_dc)]
        for c in range(n_dc):
            nc.sync.dma_start(wq_sb[c][:], wqTr[c])
        for c in range(n_dc):
            probe_src(wq_sb[c])

        # ---- k/v projections (whole NK) ----
        # note: no pool is ever closed — released-zone reuse would re-emit
        # extra waits on matmuls, which only have one HW wait slot.
        wkvp = ctx.enter_context(tc.tile_pool(name="wkv", bufs=1))
        expp = ctx.enter_context(tc.tile_pool(name="exp", bufs=3))
        outp = ctx.enter_context(tc.tile_pool(name="outT", bufs=2))
        recp = ctx.enter_context(tc.tile_pool(name="rec", bufs=1))
        avsbp = ctx.enter_context(tc.tile_pool(name="avsb", bufs=2))
        ysbp = ctx.enter_context(tc.tile_pool(name="ysb", bufs=2))
        wk_sb = [wkvp.tile([P, EL], BF16, name=f"wk{c}", tag=f"wk{c}") for c in range(n_dc)]
        wv_sb = [wkvp.tile([P, EL], BF16, name=f"wv{c}", tag=f"wv{c}") for c in range(n_dc)]
        for c in range(n_dc):
            nc.sync.dma_start(wk_sb[c][:], wkTr[c])
            nc.sync.dma_start(wv_sb[c][:], wvTr[c])
        for c in range(n_dc):
            probe_src(wk_sb[c])
            probe_src(wv_sb[c])
        # anchor for burst jb's SP dummy receivers: a late matmul of the
        # previous group, so the receivers schedule after the WAR producers
        # (the DMA handles themselves hoist too early to serve as anchors)
        kv_anchor = None
        for jb in range(n_jb):
            if kv_anchor is not None:
                dummy_sp(kv_anchor, n=2 * n_dc)
            cs = [strp.tile([P, IBS], BF16, name=f"s{c}", tag=f"s{c}") for c in range(n_dc)]
            for c in range(n_dc):
                nc.sync.dma_start(
                    cs[c][:], cTr[c][:, jb * IBS:(jb + 1) * IBS])
            # kT[e, j] += wkT[d, e].T @ cT[d, j]
            for ec in range(n_pair):
                ps = pj_ps.tile([P, IBS], F32, name="pj", tag="pj")
                for dc in range(n_dc):
                    mm = nc.tensor.matmul(
                        ps[:], wk_sb[dc][:, ec * P:(ec + 1) * P], cs[dc][:],
                        start=(dc == 0), stop=(dc == n_dc - 1))
                    if ec == n_pair - 1 and dc == n_dc - 1:
                        kv_anchor = mm
                cp = nc.vector.tensor_copy(
                    kT_sb[ec][:, jb * IBS:(jb + 1) * IBS], ps[:])
                dummy_pe(cp, n=2)
            # v[j, e] += cT[d, j].T @ wvT[d, e]
            for jl in range(IBS // P):
                jc = jb * (IBS // P) + jl
                ps = pj_ps.tile([P, EL], F32, name="pj", tag="pj")
                for dc in range(n_dc):
                    nc.tensor.matmul(
                        ps[:], cs[dc][:, jl * P:(jl + 1) * P], wv_sb[dc][:],
                        start=(dc == 0), stop=(dc == n_dc - 1))
                # one strided copy fills all 8 heads' v columns (dst stride
                # 65 skips the ones column), one more fills the ones columns
                dst = v_sb[jc][:, 0:n_lh * 65].rearrange(
                    "p (h c) -> p h c", c=65)
                src = ps[:, 0:n_lh * 64].rearrange("p (h c) -> p h c", c=64)
                cp = nc.vector.tensor_copy(dst[:, :, 0:64], src[:])
                nc.vector.tensor_copy(
                    dst[:, :, 64:65],
                    cst_sb[:, 0:n_lh].rearrange("p (h c) -> p h c", c=1))
                dummy_pe(cp, n=2)

        def qT_proj(ib, anchor=None):
            if anchor is not None:
                dummy_sp(anchor, n=2 * n_dc)
            xs = [strp.tile([P, IBS], BF16, name=f"s{c}", tag=f"s{c}") for c in range(n_dc)]
            for c in range(n_dc):
                nc.sync.dma_start(xs[c][:], xTr[c][:, ib * IBS:(ib + 1) * IBS])
            for ec in range(n_pair):
                ps = pj_ps.tile([P, IBS], F32, name="pj", tag="pj")
                for dc in range(n_dc):
                    mm = nc.tensor.matmul(
                        ps[:], wq_sb[dc][:, ec * P:(ec + 1) * P], xs[dc][:],
                        start=(dc == 0), stop=(dc == n_dc - 1))
                    if ec == n_pair - 1 and dc == 0:
                        anchor_mm = mm
                cp = nc.vector.tensor_copy(
                    qT_sb[ec][:, (ib % 2) * IBS:(ib % 2 + 1) * IBS], ps[:])
                dummy_pe(cp, n=2)
            return anchor_mm

        # dummy ACT ops: scheduled before the first exp, they provide free
        # wait slots for _fix_pe_wait_overflow to relocate surplus waits to
        # (ACT has no earlier instructions to receive them otherwise).
        scratch = constp.tile([1, 8], F32, name="scratch", tag="scratch")
        scratch2 = constp.tile([1, 8], F32, name="scratch2", tag="scratch2")

        def dummy_dve(src_ap):
            # reads what DVE just wrote: unhoistable by the scheduler, and
            # the own-proc wait is dropped post-schedule -> free wait slot
            nc.vector.tensor_copy(scratch2[0:1, 0:1], src_ap)

        def dummy_act(src_ap):
            nc.scalar.copy(scratch[0:1, 0:1], src_ap)

        for _ in range(8):
            # read a late-arriving tile so the scheduler can't hoist these
            # to the very start (receivers must follow the waits' producers)
            nc.scalar.copy(scratch[0:1, 0:1], wq_sb[0][0:1, 0:1])

        # rec rows 1-31 must be zero for the sel broadcast matmul; rows 0/32
        # are rewritten per pair.  One startup fill (DMA — walrus rejects
        # memset on float32r) keeps them zero forever.
        rec = recp.tile([33, IBS], FR, name="rec", tag="rec")
        nc.sync.dma_start(rec[:], cst[0:33, 132:132 + IBS])

        q_anchor = qT_proj(0)

        # ---- filler machinery -------------------------------------------
        # PE executes its queue in order, and the attention inner loop is
        # ACT-paced (one [128,1024] exp per chunk takes longer than the 4
        # chunk matmuls).  The qT/y projection matmuls are therefore queued
        # as "fillers" and emitted one per chunk slot inside the attention
        # loop, so PE always has independent work while waiting on exps.
        state = {"ydma": None, "q_anchor": q_anchor}
        fillers = []

        def push_qt(ib):
            # stream DMAs up front (double-buffered pool, overlaps freely)
            dummy_sp(state["q_anchor"], n=2 * n_dc)
            xs = [strp.tile([P, IBS], BF16, name=f"s{c}", tag=f"s{c}")
                  for c in range(n_dc)]
            for c in range(n_dc):
                nc.sync.dma_start(xs[c][:], xTr[c][:, ib * IBS:(ib + 1) * IBS])
            box = {}

            def op(ec, dc):
                if dc == 0:
                    box["ps"] = pj_ps.tile([P, IBS], F32, name="pj", tag="pj")
                mm = nc.tensor.matmul(
                    box["ps"][:], wq_sb[dc][:, ec * P:(ec + 1) * P],
                    xs[dc][:],
                    start=(dc == 0), stop=(dc == n_dc - 1))
                if ec == n_pair - 1 and dc == 0:
                    state["q_anchor"] = mm
                if dc == n_dc - 1:
                    cp = nc.vector.tensor_copy(
                        qT_sb[ec][:, (ib % 2) * IBS:(ib % 2 + 1) * IBS],
                        box["ps"][:])
                    dummy_pe(cp, n=2)

            for ec in range(n_pair):
                for dc in range(n_dc):
                    fillers.append(lambda ec=ec, dc=dc: op(ec, dc))

        def push_y(ot_tiles, ib):
            ib_sl = slice(ib * IBS, (ib + 1) * IBS)
            box = {}

            def op(oc, pair):
                if pair == 0:
                    box["yp"] = pj_ps.tile([P, IBS], F32, name="pj", tag="pj")
                mm = nc.tensor.matmul(
                    box["yp"][:], wo_sb[pair][:, oc * P:(oc + 1) * P],
                    ot_tiles[pair][:],
                    start=(pair == 0), stop=(pair == n_pair - 1))
                if oc == 0 and pair == n_pair - 1:
                    dummy_sp(mm, n=n_oc)
                if pair == n_pair - 1:
                    ysb = ysbp.tile([P, IBS], F32, name="y", tag="y")
                    ycp = nc.vector.tensor_copy(ysb[:], box["yp"][:])
                    dummy_sp(ycp, n=2)
                    state["ydma"] = nc.sync.dma_start(
                        yTr[oc][:, ib_sl], ysb[:])
                    dummy_dve(ysb[0:1, 0:1])

            for oc in range(n_oc):
                for pair in range(n_pair):
                    fillers.append(lambda oc=oc, pair=pair: op(oc, pair))

        # ---- main loop ---------------------------------------------------
        prev_ot = None
        for ib in range(n_ib):
            if ib + 1 < n_ib:
                push_qt(ib + 1)
            if prev_ot is not None:
                push_y(prev_ot, ib - 1)
            # Bresenham-spread the queued fillers over this ib's chunk slots
            n_slots = n_pair * n_jc
            n_ops = len(fillers)
            slot = [0]

            def fill():
                s = slot[0]
                slot[0] += 1
                take = (s + 1) * n_ops // n_slots - s * n_ops // n_slots
                for _ in range(min(take, len(fillers))):
                    fillers.pop(0)()

            ot_tiles = []
            for pair in range(n_pair):
                avA = av_ps.tile([P, IBS], F32, name="avA", tag="avA")
                avB = av_ps.tile([P, IBS], F32, name="avB", tag="avB")
                hA, hB = 2 * pair, 2 * pair + 1
                # software-pipelined: sims+exps run one key-chunk ahead of
                # the AV accumulation so PE never idles on ACT latency
                q_sl = slice((ib % 2) * IBS, (ib % 2 + 1) * IBS)
                exps = [None] * n_jc

                def sim_exp(jc, pair=pair):
                    # both heads' sims land in one 2-bank psum tile so a
                    # single [128,1024] activation computes both exps
                    sAB = sim_ps.tile([P, 2 * IBS], F32, name="sAB", tag="sAB")
                    nc.tensor.matmul(
                        sAB[:, 0:IBS], kT_sb[pair][0:64, jc * P:(jc + 1) * P],
                        qT_sb[pair][0:64, q_sl],
                        start=True, stop=True, tile_position=(0, 0))
                    nc.tensor.matmul(
                        sAB[:, IBS:2 * IBS],
                        kT_sb[pair][64:128, jc * P:(jc + 1) * P],
                        qT_sb[pair][64:128, q_sl],
                        start=True, stop=True, tile_position=(64, 0))
                    eAB = expp.tile([P, 2 * IBS], FR, name="eAB", tag="eAB")
                    nc.scalar.activation(eAB[:], sAB[:], EXP, scale=SCALE)
                    exps[jc] = eAB

                def av_acc(jc, pair=pair, hA=hA, hB=hB, avA=avA, avB=avB):
                    eAB = exps[jc]
                    st, sp = jc == 0, jc == n_jc - 1
                    # outT_unnorm[d, i] += [v_h | 1][j, :].T @ exp[j, i]:
                    # rows 0-63 = attention output, row 64 = denominator
                    nc.tensor.matmul(
                        avA[0:65, :], v_sb[jc][:, hA * 65:hA * 65 + 65],
                        eAB[:, 0:IBS],
                        start=st, stop=sp, skip_group_check=True)
                    nc.tensor.matmul(
                        avB[0:65, :], v_sb[jc][:, hB * 65:hB * 65 + 65],
                        eAB[:, IBS:2 * IBS],
                        start=st, stop=sp, skip_group_check=True)

                sim_exp(0)
                for jc in range(1, n_jc):
                    sim_exp(jc)
                    av_acc(jc - 1)
                    fill()
                av_acc(n_jc - 1)
                fill()
                # normalize: outT[d, i] = outT_unnorm[d, i] / l[i] (per head).
                # av is copied to SBUF first so its psum banks free up for the
                # next pair's accumulation (av pool is single-buffered).
                avsbA = avsbp.tile([65, IBS], F32, name="avsbA", tag="avsbA")
                avsbB = avsbp.tile([65, IBS], F32, name="avsbB", tag="avsbB")
                nc.vector.tensor_copy(avsbA[:], avA[0:65, :])
                nc.vector.tensor_copy(avsbB[:], avB[0:65, :])
                with nc.allow_low_precision(
                        reason="1/l stored as float32r for the fp32r "
                               "broadcast matmul; f32 bits either way"):
                    nc.vector.reciprocal(rec[0:1, :], avsbA[64:65, :])
                    nc.vector.reciprocal(rec[32:33, :], avsbB[64:65, :])
                bc = pj_ps.tile([P, IBS], F32, name="bc", tag="pj")
                nc.tensor.matmul(bc[:], sel[0:33, :], rec[0:33, :],
                                 start=True, stop=True, skip_group_check=True)
                ot = outp.tile([P, IBS], FR, name=f"ot{pair}", tag=f"ot{pair}")
                nc.vector.tensor_tensor(ot[0:64, :], avsbA[0:64, :],
                                        bc[0:64, :], MULT)
                nc.vector.tensor_tensor(ot[64:128, :], avsbB[0:64, :],
                                        bc[64:128, :], MULT)
                ot_tiles.append(ot)
                dummy_dve(ot[0:1, 0:1])
                dummy_dve(ot[0:1, 1:2])
                dummy_act(exps[n_jc - 1][0:1, 0:1])
            prev_ot = ot_tiles

        # last ib's output projection runs as the tail
        push_y(prev_ot, n_ib - 1)
        while fillers:
            fillers.pop(0)()

        # tail receivers for the final barrier drain's waits
        dummy_sp(state["ydma"], n=12)

    _fix_pe_wait_overflow(nc)
    return nc


def _fix_pe_wait_overflow(nc):
    """Each hardware instruction has a single sync-wait slot (walrus: 'Too
    many sync wait commands').  Normalize every instruction to at most one
    wait in four phases:
      1. merge same-semaphore waits to the max value;
      2. drop own-proc waits (compute engines execute strictly in order);
      3. drop waits already implied by an earlier wait on the same engine
         (per-engine observed-tick tracking — Tile's emission is not
         transitive across instructions);
      4. move remaining surplus waits backwards onto an earlier same-engine
         instruction with a free slot (sem values are monotonic, so waiting
         earlier is strictly stronger; the scheduled block order is a
         topological order, so any receiver after the wait's producer
         cannot deadlock).
    """
    import bisect

    SKIP = ("InstISA", "InstEventSemaphore", "InstTriggerDma", "InstNoOp")
    ENG_SEM = {"EngineType.PE": "PE_", "EngineType.DVE": "DVE_",
               "EngineType.Activation": "Activation_",
               "EngineType.Pool": "Pool_", "EngineType.SP": "SP_"}

    # flatten all basic blocks (they execute sequentially) so cross-block
    # deps (e.g. the tail barrier drain) can relocate into earlier blocks
    if True:
        insts = []
        for blk in nc.m.functions[0].blocks:
            insts.extend(blk.instructions)
        cum = {}
        prod = {}  # sem name -> (cumulative values, instruction indices)
        for idx, i in enumerate(insts):
            si = i.sync_info
            if not si:
                continue
            for u in si.on_update:
                n = str(getattr(u, "ant_name", ""))
                v = cum.get(n, 0) + (u.update_value or 1)
                cum[n] = v
                vs, ids = prod.setdefault(n, ([], []))
                vs.append(v)
                ids.append(idx)

        def producer_idx(name, value):
            vs, ids = prod.get(name, ([], []))
            k = bisect.bisect_left(vs, value)
            return ids[k] if k < len(vs) else len(insts)

        def eng_of(i):
            return str(getattr(i, "engine", ""))

        def waits_of(i):
            si = i.sync_info
            return list(si.on_wait) if si else []

        def set_waits(i, ws):
            if i.sync_info is None:
                i.sync_info = mybir.SyncInfo(on_wait=ws, on_update=[])
            else:
                i.sync_info.on_wait = ws

        # phase 1+2: merge same-sem; drop own-proc waits
        for i in insts:
            if type(i).__name__ in SKIP:
                continue
            ws = waits_of(i)
            if not ws:
                continue
            best = {}
            for w in ws:
                n = str(getattr(w, "ant_name", ""))
                if n not in best or best[n].wait_value < w.wait_value:
                    best[n] = w
            own = ENG_SEM.get(eng_of(i))
            if own is not None and type(i).__name__ != "InstDMACopy":
                for n in list(best):
                    if n.startswith(own):
                        q = producer_idx(n, best[n].wait_value)
                        if q < len(insts) and eng_of(insts[q]) == eng_of(i) \
                                and type(insts[q]).__name__ not in SKIP:
                            del best[n]
            if len(best) != len(ws):
                set_waits(i, list(best.values()))

        def observed_sweep():
            # phase 3: per-engine observed ticks; drop implied waits
            obs = {}
            for i in insts:
                if type(i).__name__ in SKIP:
                    continue
                ws = waits_of(i)
                if not ws:
                    continue
                e = obs.setdefault(eng_of(i), {})
                kept = []
                for w in ws:
                    n = str(getattr(w, "ant_name", ""))
                    if e.get(n, -1) >= w.wait_value:
                        continue
                    kept.append(w)
                    e[n] = w.wait_value
                if len(kept) != len(ws):
                    set_waits(i, kept)

        observed_sweep()

        # phase 4: relocate surplus waits backwards (with cascading:
        # a receiver holding one wait can itself be freed by pushing its
        # wait further back, as long as every placement stays after the
        # corresponding producer)
        def can_receive(r_idx):
            t = type(insts[r_idx]).__name__
            return t == "InstNoOp" or t not in SKIP

        def place(w, lo, hi, eng, depth):
            """Place wait w on some same-engine instruction in (lo, hi).
            Returns True on success."""
            if depth <= 0:
                return False
            n = str(getattr(w, "ant_name", ""))
            for r in range(hi - 1, lo, -1):
                cand = insts[r]
                if eng_of(cand) != eng or not can_receive(r):
                    continue
                cw = waits_of(cand)
                if len(cw) == 0:
                    set_waits(cand, [w])
                    return True
                if len(cw) == 1:
                    cn = str(getattr(cw[0], "ant_name", ""))
                    if cn == n:
                        # same-sem: raising to max covers both
                        if cw[0].wait_value < w.wait_value:
                            set_waits(cand, [w])
                        return True
            # cascade: free a candidate by pushing its wait further back
            for r in range(hi - 1, lo, -1):
                cand = insts[r]
                if eng_of(cand) != eng or not can_receive(r):
                    continue
                cw = waits_of(cand)
                if len(cw) != 1 or type(cand).__name__ == "InstNoOp":
                    continue
                cq = producer_idx(str(getattr(cw[0], "ant_name", "")),
                                  cw[0].wait_value)
                if place(cw[0], cq, r, eng, depth - 1):
                    set_waits(cand, [w])
                    return True
            return False

        for idx, i in enumerate(insts):
            if type(i).__name__ in SKIP:
                continue
            ws = waits_of(i)
            if len(ws) <= 1:
                continue
            eng = eng_of(i)
            ws.sort(key=lambda w: producer_idx(
                str(getattr(w, "ant_name", "")), w.wait_value))
            remaining = list(ws)
            progress = True
            while len(remaining) > 1 and progress:
                progress = False
                for w in list(remaining):
                    if len(remaining) <= 1:
                        break
                    q = producer_idx(str(getattr(w, "ant_name", "")),
                                     w.wait_value)
                    if place(w, q, idx, eng, 4):
                        remaining.remove(w)
                        progress = True
                        break
            assert len(remaining) <= 1, (
                f"{i.name} ({eng}): cannot reduce waits "
                f"{[(str(w.ant_name), w.wait_value) for w in remaining]} "
                f"producers "
                f"{[producer_idx(str(w.ant_name), w.wait_value) for w in remaining]} "
                f"at idx {idx}")
            set_waits(i, remaining)

        observed_sweep()


_CACHE = {}


def _get_module():
    if "nc" not in _CACHE:
        _CACHE["nc"] = build_module()
    return _CACHE["nc"]


def make_in_maps(x, context, Wq, Wk, Wv, Wo):
    import ml_dtypes
    bf = ml_dtypes.bfloat16
    x = np.asarray(x, np.float32).astype(bf)
    context = np.asarray(context, np.float32).astype(bf)
    Wq = np.asarray(Wq, np.float32).astype(bf)
    Wk = np.asarray(Wk, np.float32).astype(bf)
    Wv = np.asarray(Wv, np.float32).astype(bf)
    Wo = np.asarray(Wo, np.float32)
    cst = np.zeros((P, 644), np.float32)
    cst[:, 0:16] = 1.0
    cstf = np.zeros((33, P), np.float32)
    cstf[0, 0:64] = 1.0
    cstf[32, 64:128] = 1.0
    in_maps = []
    for c in range(N_CORES):
        b, g = divmod(c, 2)
        sl = slice(g * EL, (g + 1) * EL)
        in_maps.append({
            "xT": np.ascontiguousarray(x[b].T),
            "cT": np.ascontiguousarray(context[b].T),
            "wqT": np.ascontiguousarray(Wq[sl].T),
            "wkT": np.ascontiguousarray(Wk[sl].T),
            "wvT": np.ascontiguousarray(Wv[sl].T),
            "woT": np.ascontiguousarray(Wo[:, sl].T),
            "cst": cst,
            "cstf": cstf,
        })
    return in_maps


def gather_output(results, bo):
    bo = np.asarray(bo, np.float32)
    y = np.empty((B, NQ, D), np.float32)
    for b in range(B):
        y[b] = (results[2 * b]["yT"] + results[2 * b + 1]["yT"]).T + bo
    return y


def kernel(x, context, Wq, Wk, Wv, Wo, bo):
    nc = _get_module()
    in_maps = make_in_maps(x, context, Wq, Wk, Wv, Wo)
    res = run_bass_kernel_spmd(nc, in_maps, core_ids=list(range(N_CORES)))
    return gather_output(res.results, bo)



# revision 24
# speedup vs baseline: 1.3404x; 1.0639x over previous
"""Trainium2 Bass kernel: CrossAttention  (B=4, N=M=2048, D=1024, 16 heads x 64).

Sharding: 8 cores <- (batch, head-half): core c handles batch c//2, heads
(c%2)*8 .. (c%2)*8+8 (inner dims e = (c%2)*512 .. +512).  Each core computes
q/k/v projections for its slice, attention for its 8 heads, and the partial
output projection  yT_partial = WoT_loc.T @ outT_loc.  Host sums the two
partials per batch and adds the bias.

Device layout: transposed throughout (inner/contract dim on partitions):
  xT,cT [D, N], wqT/wkT/wvT [D, EL], woT [EL, D]; output yT [D, N].
All matmul operands are float32r (full PE rate at moving free-dim >= 256,
near-fp32 precision; measured end-to-end rel err ~1.4e-4).  Softmax is
max-free (logit scale ~0.4, safe for exp).  V is stored per head as
[v_h | ones] so one [65 x 512] matmul accumulates both the attention
output (rows 0-63) and the softmax denominator (row 64); normalization is
applied to the 64-row attention output via a K=33 broadcast matmul before
the output projection.  The QK sims row-pack two 64-dim heads into the
128x128 PE array (row tiling is the only fp32r-legal array packing; all
matmul outputs must start at psum partition 0).

Walrus/TRN2 constraint handled by _fix_pe_wait_overflow: every hardware
instruction has a single sync-wait slot, so the build post-processes the
scheduled module to merge/drop/relocate waits, helped by pinned NOP
wait-receivers emitted next to DMA bursts.
"""

import numpy as np
from contextlib import ExitStack

import concourse.bass as bass
import concourse.tile as tile
from concourse import mybir
from concourse.bass_utils import run_bass_kernel_spmd
from concourse.tile_rust import add_dep_helper

P = 128
FR = mybir.dt.float32r
F32 = mybir.dt.float32
BF16 = mybir.dt.bfloat16
EXP = mybir.ActivationFunctionType.Exp
MULT = mybir.AluOpType.mult

# problem dims (hardcoded per the harness contract)
B, NQ, NK, D = 4, 2048, 2048, 1024
HEADS, DIM_HEAD = 16, 64
INNER = HEADS * DIM_HEAD
EL = 512  # inner dims per core (8 heads)
SCALE = DIM_HEAD ** -0.5
IBS = 512  # query/key block size (psum bank free size)
N_CORES = 8


def build_module(D=D, NQ=NQ, NK=NK, EL=EL, trace_sim=False):
    n_ib = NQ // IBS   # query blocks
    n_jb = NK // IBS   # key blocks (projection granularity)
    n_jc = NK // P     # key chunks (attention contraction granularity)
    n_dc = D // P      # model-dim chunks
    n_pair = EL // P   # head pairs
    n_oc = D // P      # output-dim chunks

    nc = bass.Bass("TRN2", target_bir_lowering=False, debug=False)
    # projection operands (x/c streams + q/k/v weights) are bf16: the PE rate
    # is identical, DMA bytes and SBUF halve, and the precision cost (~0.3%
    # per projection) is far inside the 2e-2 budget.
    xT = nc.dram_tensor("xT", [D, NQ], BF16, kind="ExternalInput").ap()
    cT = nc.dram_tensor("cT", [D, NK], BF16, kind="ExternalInput").ap()
    wqT = nc.dram_tensor("wqT", [D, EL], BF16, kind="ExternalInput").ap()
    wkT = nc.dram_tensor("wkT", [D, EL], BF16, kind="ExternalInput").ap()
    wvT = nc.dram_tensor("wvT", [D, EL], BF16, kind="ExternalInput").ap()
    woT = nc.dram_tensor("woT", [EL, D], FR, kind="ExternalInput").ap()
    yT = nc.dram_tensor("yT", [D, NQ], F32, kind="ExternalOutput").ap()

    xTr = xT.rearrange("(c p) n -> c p n", p=P)
    cTr = cT.rearrange("(c p) n -> c p n", p=P)
    wqTr = wqT.rearrange("(c p) e -> c p e", p=P)
    wkTr = wkT.rearrange("(c p) e -> c p e", p=P)
    wvTr = wvT.rearrange("(c p) e -> c p e", p=P)
    woTr = woT.rearrange("(c p) o -> c p o", p=P)
    yTr = yT.rearrange("(c p) n -> c p n", p=P)

    with tile.TileContext(nc, trace_sim=trace_sim) as tc, ExitStack() as ctx:

        constp = ctx.enter_context(tc.tile_pool(name="const", bufs=1))
        wop = ctx.enter_context(tc.tile_pool(name="wo", bufs=1))
        qtp = ctx.enter_context(tc.tile_pool(name="qt", bufs=1))
        ktp = ctx.enter_context(tc.tile_pool(name="kt", bufs=1))
        vp = ctx.enter_context(tc.tile_pool(name="v", bufs=1))
        wqp = ctx.enter_context(tc.tile_pool(name="wq", bufs=1))
        # double-buffered so the next block's x/c DMAs overlap this block's
        # projection matmuls
        strp = ctx.enter_context(tc.tile_pool(name="stream", bufs=2))

        pj_ps = ctx.enter_context(tc.tile_pool(name="pjps", bufs=2, space="PSUM"))
        sim_ps = ctx.enter_context(tc.tile_pool(name="simps", bufs=2, space="PSUM"))
        av_ps = ctx.enter_context(tc.tile_pool(name="avps", bufs=1, space="PSUM"))

        # constants come in via DMA (walrus rejects memset on float32r):
        # cols 0..7 = ones (v ones-fill + AV denominator rows); cols 8..135 =
        # sel rows... sel is its own FR tensor below.
        cst = nc.dram_tensor("cst", [P, 644], FR, kind="ExternalInput").ap()
        cstf = nc.dram_tensor("cstf", [33, P], FR, kind="ExternalInput").ap()
        cst_sb = constp.tile([P, 644], FR, name="cst", tag="cst")
        sel = constp.tile([33, P], FR, name="selfr", tag="selfr")
        # startup-only probe target: borrows the avA slot (released before
        # attention starts; slot reuse is same-engine and needs no sems)
        probe_ps = av_ps.tile([P, IBS], F32, name="prb", tag="avA")

        # Each PE matmul has a single HW wait slot, and fp32r matmuls are
        # self-loading (no separate ldweights to carry a second wait).
        # probe_src makes PE observe a freshly-DMA'd weight tile's queue
        # tick up front, so later matmuls reading (weights, activations)
        # carry only the activation-chunk queue tick.
        def probe_src(src):
            # K=1, dst [32, 64] at base 0 (a 1x1 dst fails walrus ISA checks)
            nc.tensor.matmul(probe_ps[0:32, 0:64], src[0:1, 0:32],
                             src[0:1, 0:64],
                             start=True, stop=True, skip_group_check=True)

        # SP-stream dummies: dependency-free sequencer NOPs pinned after
        # `anchor` via a sync-free scheduling dep, so they sit between a
        # surplus wait's producer and the real DMA that needs to shed it —
        # free wait slots for _fix_pe_wait_overflow.
        def dummy_sp(anchor, n=1):
            for _ in range(n):
                h = nc.sync.nop()
                if anchor is not None:
                    add_dep_helper(h.ins, anchor.ins, sync=False,
                                   reason="pin SP dummy wait-receiver")

        def dummy_pe(anchor, n=1):
            for _ in range(n):
                h = nc.tensor.nop()
                if anchor is not None:
                    add_dep_helper(h.ins, anchor.ins, sync=False,
                                   reason="pin PE dummy wait-receiver")

        # ---- pools & persistent tiles ----
        # note: no pool is ever closed — released-zone reuse would re-emit
        # extra waits on matmuls, which only have one HW wait slot.
        wkvp = ctx.enter_context(tc.tile_pool(name="wkv", bufs=1))
        expp = ctx.enter_context(tc.tile_pool(name="exp", bufs=3))
        outp = ctx.enter_context(tc.tile_pool(name="outT", bufs=2))
        recp = ctx.enter_context(tc.tile_pool(name="rec", bufs=1))
        avsbp = ctx.enter_context(tc.tile_pool(name="avsb", bufs=2))
        ysbp = ctx.enter_context(tc.tile_pool(name="ysb", bufs=4))

        wo_sb = [wop.tile([P, D], FR, name=f"wo{c}", tag=f"wo{c}") for c in range(n_pair)]
        # qT is double-buffered by query block (half = ib%2) to save SBUF
        qT_sb = [qtp.tile([P, 2 * IBS], FR, name=f"q{c}", tag=f"q{c}") for c in range(n_pair)]
        kT_sb = [ktp.tile([P, NK], FR, name=f"k{c}", tag=f"k{c}") for c in range(n_pair)]
        # v is stored per head as [v_h | ones] (65 cols): the ones column
        # makes the AV matmul also produce the softmax denominator in row 64
        n_lh = 2 * n_pair
        v_sb = [vp.tile([P, n_lh * 65], FR, name=f"v{j}", tag=f"v{j}")
                for j in range(n_jc)]
        wq_sb = [wqp.tile([P, EL], BF16, name=f"wq{c}", tag=f"wq{c}") for c in range(n_dc)]
        wk_sb = [wkvp.tile([P, EL], BF16, name=f"wk{c}", tag=f"wk{c}") for c in range(n_dc)]
        wv_sb = [wkvp.tile([P, EL], BF16, name=f"wv{c}", tag=f"wv{c}") for c in range(n_dc)]

        # ---- k/v projections (whole NK) ----
        # DMA order is chosen so the first kv block can start ASAP on the
        # (serialized) DMA engines: wk, then the first context block, then
        # wv (the v matmuls run after the k matmuls); cst/wq/sel load during
        # jb0's compute and wo during jb1's.
        for c in range(n_dc):
            nc.sync.dma_start(wk_sb[c][:], wkTr[c])

        # anchor for burst jb's SP dummy receivers: a late matmul of the
        # previous group, so the receivers schedule after the WAR producers
        # (the DMA handles themselves hoist too early to serve as anchors)
        kv_anchor = [None]

        def kv_dma(jb):
            if kv_anchor[0] is not None:
                dummy_sp(kv_anchor[0], n=2 * n_dc)
            cs = [strp.tile([P, IBS], BF16, name=f"s{c}", tag=f"s{c}") for c in range(n_dc)]
            for c in range(n_dc):
                nc.sync.dma_start(
                    cs[c][:], cTr[c][:, jb * IBS:(jb + 1) * IBS])
            return cs

        def kv_block(jb, cs):
            # kT[e, j] += wkT[d, e].T @ cT[d, j]
            for ec in range(n_pair):
                ps = pj_ps.tile([P, IBS], F32, name="pj", tag="pj")
                for dc in range(n_dc):
                    mm = nc.tensor.matmul(
                        ps[:], wk_sb[dc][:, ec * P:(ec + 1) * P], cs[dc][:],
                        start=(dc == 0), stop=(dc == n_dc - 1))
                    if ec == n_pair - 1 and dc == n_dc - 1:
                        kv_anchor[0] = mm
                cp = nc.vector.tensor_copy(
                    kT_sb[ec][:, jb * IBS:(jb + 1) * IBS], ps[:])
                dummy_pe(cp, n=2)
            # v[j, e] += cT[d, j].T @ wvT[d, e]
            for jl in range(IBS // P):
                jc = jb * (IBS // P) + jl
                ps = pj_ps.tile([P, EL], F32, name="pj", tag="pj")
                for dc in range(n_dc):
                    nc.tensor.matmul(
                        ps[:], cs[dc][:, jl * P:(jl + 1) * P], wv_sb[dc][:],
                        start=(dc == 0), stop=(dc == n_dc - 1))
                # one strided copy fills all 8 heads' v columns (dst stride
                # 65 skips the ones column), one more fills the ones columns
                dst = v_sb[jc][:, 0:n_lh * 65].rearrange(
                    "p (h c) -> p h c", c=65)
                src = ps[:, 0:n_lh * 64].rearrange("p (h c) -> p h c", c=64)
                cp = nc.vector.tensor_copy(dst[:, :, 0:64], src[:])
                nc.vector.tensor_copy(
                    dst[:, :, 64:65],
                    cst_sb[:, 0:n_lh].rearrange("p (h c) -> p h c", c=1))
                dummy_pe(cp, n=2)

        cs0 = kv_dma(0)
        # wv and cst land on the DMA engines behind wk+cs0; both are only
        # read by jb0's v-phase, which runs after the k-phase compute.
        for c in range(n_dc):
            nc.sync.dma_start(wv_sb[c][:], wvTr[c])
        nc.sync.dma_start(cst_sb[:], cst[:])
        for c in range(n_dc):
            probe_src(wk_sb[c])
            probe_src(wv_sb[c])
        probe_src(cst_sb)

        kv_block(0, cs0)

        # deferred loads: needed only after jb0's compute window
        for c in range(n_dc):
            nc.sync.dma_start(wq_sb[c][:], wqTr[c])
        for c in range(n_dc):
            probe_src(wq_sb[c])
        nc.sync.dma_start(sel[:], cstf[:])

        cs1 = kv_dma(1)
        kv_block(1, cs1)

        for c in range(n_pair):
            nc.sync.dma_start(wo_sb[c][:], woTr[c])
        for c in range(n_pair):
            probe_src(wo_sb[c])

        for jb in range(2, n_jb):
            kv_block(jb, kv_dma(jb))

        def qT_proj(ib, anchor=None):
            if anchor is not None:
                dummy_sp(anchor, n=2 * n_dc)
            xs = [strp.tile([P, IBS], BF16, name=f"s{c}", tag=f"s{c}") for c in range(n_dc)]
            for c in range(n_dc):
                nc.sync.dma_start(xs[c][:], xTr[c][:, ib * IBS:(ib + 1) * IBS])
            for ec in range(n_pair):
                ps = pj_ps.tile([P, IBS], F32, name="pj", tag="pj")
                for dc in range(n_dc):
                    mm = nc.tensor.matmul(
                        ps[:], wq_sb[dc][:, ec * P:(ec + 1) * P], xs[dc][:],
                        start=(dc == 0), stop=(dc == n_dc - 1))
                    if ec == n_pair - 1 and dc == 0:
                        anchor_mm = mm
                cp = nc.vector.tensor_copy(
                    qT_sb[ec][:, (ib % 2) * IBS:(ib % 2 + 1) * IBS], ps[:])
                dummy_pe(cp, n=2)
            return anchor_mm

        # dummy ACT ops: scheduled before the first exp, they provide free
        # wait slots for _fix_pe_wait_overflow to relocate surplus waits to
        # (ACT has no earlier instructions to receive them otherwise).
        scratch = constp.tile([1, 8], F32, name="scratch", tag="scratch")
        scratch2 = constp.tile([1, 8], F32, name="scratch2", tag="scratch2")

        def dummy_dve(src_ap):
            # reads what DVE just wrote: unhoistable by the scheduler, and
            # the own-proc wait is dropped post-schedule -> free wait slot
            nc.vector.tensor_copy(scratch2[0:1, 0:1], src_ap)

        def dummy_act(src_ap):
            nc.scalar.copy(scratch[0:1, 0:1], src_ap)

        for _ in range(8):
            # read a late-arriving tile so the scheduler can't hoist these
            # to the very start (receivers must follow the waits' producers)
            nc.scalar.copy(scratch[0:1, 0:1], wq_sb[0][0:1, 0:1])

        # rec rows 1-31 must be zero for the sel broadcast matmul; rows 0/32
        # are rewritten per pair.  One startup fill (DMA — walrus rejects
        # memset on float32r) keeps them zero forever.
        rec = recp.tile([33, IBS], FR, name="rec", tag="rec")
        nc.sync.dma_start(rec[:], cst[0:33, 132:132 + IBS])

        q_anchor = qT_proj(0)

        # ---- filler machinery -------------------------------------------
        # PE executes its queue in order, and the attention inner loop is
        # ACT-paced (one [128,1024] exp per chunk takes longer than the 4
        # chunk matmuls).  The qT/y projection matmuls are therefore queued
        # as "fillers" and emitted one per chunk slot inside the attention
        # loop, so PE always has independent work while waiting on exps.
        state = {"ydma": None, "q_anchor": q_anchor}
        fillers = []

        def push_qt(ib):
            # stream DMAs up front (double-buffered pool, overlaps freely)
            dummy_sp(state["q_anchor"], n=2 * n_dc)
            xs = [strp.tile([P, IBS], BF16, name=f"s{c}", tag=f"s{c}")
                  for c in range(n_dc)]
            for c in range(n_dc):
                nc.sync.dma_start(xs[c][:], xTr[c][:, ib * IBS:(ib + 1) * IBS])
            box = {}

            def op(ec, dc):
                if dc == 0:
                    box["ps"] = pj_ps.tile([P, IBS], F32, name="pj", tag="pj")
                mm = nc.tensor.matmul(
                    box["ps"][:], wq_sb[dc][:, ec * P:(ec + 1) * P],
                    xs[dc][:],
                    start=(dc == 0), stop=(dc == n_dc - 1))
                if ec == n_pair - 1 and dc == 0:
                    state["q_anchor"] = mm
                if dc == n_dc - 1:
                    cp = nc.vector.tensor_copy(
                        qT_sb[ec][:, (ib % 2) * IBS:(ib % 2 + 1) * IBS],
                        box["ps"][:])
                    dummy_pe(cp, n=2)

            for ec in range(n_pair):
                for dc in range(n_dc):
                    fillers.append(lambda ec=ec, dc=dc: op(ec, dc))

        def push_y(ot_tiles, ib):
            ib_sl = slice(ib * IBS, (ib + 1) * IBS)
            box = {}

            def op(oc, pair):
                if pair == 0:
                    box["yp"] = pj_ps.tile([P, IBS], F32, name="pj", tag="pj")
                mm = nc.tensor.matmul(
                    box["yp"][:], wo_sb[pair][:, oc * P:(oc + 1) * P],
                    ot_tiles[pair][:],
                    start=(pair == 0), stop=(pair == n_pair - 1))
                if oc == 0 and pair == n_pair - 1:
                    dummy_sp(mm, n=n_oc)
                if pair == n_pair - 1:
                    ysb = ysbp.tile([P, IBS], F32, name="y", tag="y")
                    ycp = nc.vector.tensor_copy(ysb[:], box["yp"][:])
                    dummy_sp(ycp, n=2)
                    state["ydma"] = nc.sync.dma_start(
                        yTr[oc][:, ib_sl], ysb[:])
                    dummy_dve(ysb[0:1, 0:1])

            for oc in range(n_oc):
                for pair in range(n_pair):
                    fillers.append(lambda oc=oc, pair=pair: op(oc, pair))

        # ---- main loop ---------------------------------------------------
        prev_ot = None
        pending_norm = [None]
        for ib in range(n_ib):
            if ib + 1 < n_ib:
                push_qt(ib + 1)
            if prev_ot is not None:
                push_y(prev_ot, ib - 1)
            # Bresenham-spread the queued fillers over this ib's fill slots
            n_slots = n_pair * (n_jc + 3)
            n_ops = len(fillers)
            slot = [0]

            def fill():
                s = slot[0]
                slot[0] += 1
                take = (s + 1) * n_ops // n_slots - s * n_ops // n_slots
                for _ in range(min(take, len(fillers))):
                    fillers.pop(0)()

            ot_tiles = []
            for pair in range(n_pair):
                avA = av_ps.tile([P, IBS], F32, name="avA", tag="avA")
                avB = av_ps.tile([P, IBS], F32, name="avB", tag="avB")
                hA, hB = 2 * pair, 2 * pair + 1
                # software-pipelined: sims+exps run one key-chunk ahead of
                # the AV accumulation so PE never idles on ACT latency
                q_sl = slice((ib % 2) * IBS, (ib % 2 + 1) * IBS)
                exps = [None] * n_jc

                def sim_exp(jc, pair=pair):
                    # both heads' sims land in one 2-bank psum tile so a
                    # single [128,1024] activation computes both exps
                    sAB = sim_ps.tile([P, 2 * IBS], F32, name="sAB", tag="sAB")
                    nc.tensor.matmul(
                        sAB[:, 0:IBS], kT_sb[pair][0:64, jc * P:(jc + 1) * P],
                        qT_sb[pair][0:64, q_sl],
                        start=True, stop=True, tile_position=(0, 0))
                    nc.tensor.matmul(
                        sAB[:, IBS:2 * IBS],
                        kT_sb[pair][64:128, jc * P:(jc + 1) * P],
                        qT_sb[pair][64:128, q_sl],
                        start=True, stop=True, tile_position=(64, 0))
                    eAB = expp.tile([P, 2 * IBS], FR, name="eAB", tag="eAB")
                    nc.scalar.activation(eAB[:], sAB[:], EXP, scale=SCALE)
                    exps[jc] = eAB

                def av_acc(jc, pair=pair, hA=hA, hB=hB, avA=avA, avB=avB):
                    eAB = exps[jc]
                    st, sp = jc == 0, jc == n_jc - 1
                    # outT_unnorm[d, i] += [v_h | 1][j, :].T @ exp[j, i]:
                    # rows 0-63 = attention output, row 64 = denominator
                    nc.tensor.matmul(
                        avA[0:65, :], v_sb[jc][:, hA * 65:hA * 65 + 65],
                        eAB[:, 0:IBS],
                        start=st, stop=sp, skip_group_check=True)
                    nc.tensor.matmul(
                        avB[0:65, :], v_sb[jc][:, hB * 65:hB * 65 + 65],
                        eAB[:, IBS:2 * IBS],
                        start=st, stop=sp, skip_group_check=True)

                # depth-2 software pipeline: AV lags sims/exps by two chunks,
                # giving the previous pair's avsb copies time to release the
                # single-buffered av banks before this pair's first av_acc.
                # Extra fill slots sit at the pair seams, where PE would
                # otherwise stall while ACT catches up on the last exps.
                sim_exp(0)
                fill()
                if pending_norm[0] is not None:
                    # previous pair's bc matmul + muls, emitted here so the
                    # PE-blocking bc doesn't stall this pair's first sims
                    # while the DVE recips run
                    pending_norm[0]()
                    pending_norm[0] = None
                fill()
                sim_exp(1)
                for jc in range(2, n_jc):
                    sim_exp(jc)
                    av_acc(jc - 2)
                    fill()
                fill()
                av_acc(n_jc - 2)
                fill()
                av_acc(n_jc - 1)
                # normalize: outT[d, i] = outT_unnorm[d, i] / l[i] (per head).
                # av is copied to SBUF first so its psum banks free up for the
                # next pair's accumulation (av pool is single-buffered).
                avsbA = avsbp.tile([65, IBS], F32, name="avsbA", tag="avsbA")
                avsbB = avsbp.tile([65, IBS], F32, name="avsbB", tag="avsbB")
                nc.vector.tensor_copy(avsbA[:], avA[0:65, :])
                nc.vector.tensor_copy(avsbB[:], avB[0:65, :])
                with nc.allow_low_precision(
                        reason="1/l stored as float32r for the fp32r "
                               "broadcast matmul; f32 bits either way"):
                    nc.vector.reciprocal(rec[0:1, :], avsbA[64:65, :])
                    nc.vector.reciprocal(rec[32:33, :], avsbB[64:65, :])
                fill()
                ot = outp.tile([P, IBS], FR, name=f"ot{pair}", tag=f"ot{pair}")
                last_eAB = exps[n_jc - 1]

                def finish_norm(avsbA=avsbA, avsbB=avsbB, ot=ot,
                                last_eAB=last_eAB):
                    bc = pj_ps.tile([P, IBS], F32, name="bc", tag="pj")
                    nc.tensor.matmul(bc[:], sel[0:33, :], rec[0:33, :],
                                     start=True, stop=True,
                                     skip_group_check=True)
                    nc.vector.tensor_tensor(ot[0:64, :], avsbA[0:64, :],
                                            bc[0:64, :], MULT)
                    nc.vector.tensor_tensor(ot[64:128, :], avsbB[0:64, :],
                                            bc[64:128, :], MULT)
                    dummy_dve(ot[0:1, 0:1])
                    dummy_dve(ot[0:1, 1:2])
                    dummy_act(last_eAB[0:1, 0:1])

                pending_norm[0] = finish_norm
                ot_tiles.append(ot)
            prev_ot = ot_tiles

        # flush the last pair's norm, then run the last ib's output
        # projection as the tail
        pending_norm[0]()
        pending_norm[0] = None
        push_y(prev_ot, n_ib - 1)
        while fillers:
            fillers.pop(0)()

        # tail receivers for the final barrier drain's waits
        dummy_sp(state["ydma"], n=12)

    _fix_pe_wait_overflow(nc)
    return nc


def _fix_pe_wait_overflow(nc):
    """Each hardware instruction has a single sync-wait slot (walrus: 'Too
    many sync wait commands').  Normalize every instruction to at most one
    wait in four phases:
      1. merge same-semaphore waits to the max value;
      2. drop own-proc waits (compute engines execute strictly in order);
      3. drop waits already implied by an earlier wait on the same engine
         (per-engine observed-tick tracking — Tile's emission is not
         transitive across instructions);
      4. move remaining surplus waits backwards onto an earlier same-engine
         instruction with a free slot (sem values are monotonic, so waiting
         earlier is strictly stronger; the scheduled block order is a
         topological order, so any receiver after the wait's producer
         cannot deadlock).
    """
    import bisect

    SKIP = ("InstISA", "InstEventSemaphore", "InstTriggerDma", "InstNoOp")
    ENG_SEM = {"EngineType.PE": "PE_", "EngineType.DVE": "DVE_",
               "EngineType.Activation": "Activation_",
               "EngineType.Pool": "Pool_", "EngineType.SP": "SP_"}

    # flatten all basic blocks (they execute sequentially) so cross-block
    # deps (e.g. the tail barrier drain) can relocate into earlier blocks
    if True:
        insts = []
        for blk in nc.m.functions[0].blocks:
            insts.extend(blk.instructions)
        cum = {}
        prod = {}  # sem name -> (cumulative values, instruction indices)
        for idx, i in enumerate(insts):
            si = i.sync_info
            if not si:
                continue
            for u in si.on_update:
                n = str(getattr(u, "ant_name", ""))
                v = cum.get(n, 0) + (u.update_value or 1)
                cum[n] = v
                vs, ids = prod.setdefault(n, ([], []))
                vs.append(v)
                ids.append(idx)

        def producer_idx(name, value):
            vs, ids = prod.get(name, ([], []))
            k = bisect.bisect_left(vs, value)
            return ids[k] if k < len(vs) else len(insts)

        def eng_of(i):
            return str(getattr(i, "engine", ""))

        def waits_of(i):
            si = i.sync_info
            return list(si.on_wait) if si else []

        def set_waits(i, ws):
            if i.sync_info is None:
                i.sync_info = mybir.SyncInfo(on_wait=ws, on_update=[])
            else:
                i.sync_info.on_wait = ws

        # phase 1+2: merge same-sem; drop own-proc waits
        for i in insts:
            if type(i).__name__ in SKIP:
                continue
            ws = waits_of(i)
            if not ws:
                continue
            best = {}
            for w in ws:
                n = str(getattr(w, "ant_name", ""))
                if n not in best or best[n].wait_value < w.wait_value:
                    best[n] = w
            own = ENG_SEM.get(eng_of(i))
            if own is not None and type(i).__name__ != "InstDMACopy":
                for n in list(best):
                    if n.startswith(own):
                        q = producer_idx(n, best[n].wait_value)
                        if q < len(insts) and eng_of(insts[q]) == eng_of(i) \
                                and type(insts[q]).__name__ not in SKIP:
                            del best[n]
            if len(best) != len(ws):
                set_waits(i, list(best.values()))

        def observed_sweep():
            # phase 3: per-engine observed ticks; drop implied waits
            obs = {}
            for i in insts:
                if type(i).__name__ in SKIP:
                    continue
                ws = waits_of(i)
                if not ws:
                    continue
                e = obs.setdefault(eng_of(i), {})
                kept = []
                for w in ws:
                    n = str(getattr(w, "ant_name", ""))
                    if e.get(n, -1) >= w.wait_value:
                        continue
                    kept.append(w)
                    e[n] = w.wait_value
                if len(kept) != len(ws):
                    set_waits(i, kept)

        observed_sweep()

        # phase 4: relocate surplus waits backwards (with cascading:
        # a receiver holding one wait can itself be freed by pushing its
        # wait further back, as long as every placement stays after the
        # corresponding producer)
        def can_receive(r_idx):
            t = type(insts[r_idx]).__name__
            return t == "InstNoOp" or t not in SKIP

        def place(w, lo, hi, eng, depth):
            """Place wait w on some same-engine instruction in (lo, hi).
            Returns True on success."""
            if depth <= 0:
                return False
            n = str(getattr(w, "ant_name", ""))
            for r in range(hi - 1, lo, -1):
                cand = insts[r]
                if eng_of(cand) != eng or not can_receive(r):
                    continue
                cw = waits_of(cand)
                if len(cw) == 0:
                    set_waits(cand, [w])
                    return True
                if len(cw) == 1:
                    cn = str(getattr(cw[0], "ant_name", ""))
                    if cn == n:
                        # same-sem: raising to max covers both
                        if cw[0].wait_value < w.wait_value:
                            set_waits(cand, [w])
                        return True
            # cascade: free a candidate by pushing its wait further back
            for r in range(hi - 1, lo, -1):
                cand = insts[r]
                if eng_of(cand) != eng or not can_receive(r):
                    continue
                cw = waits_of(cand)
                if len(cw) != 1 or type(cand).__name__ == "InstNoOp":
                    continue
                cq = producer_idx(str(getattr(cw[0], "ant_name", "")),
                                  cw[0].wait_value)
                if place(cw[0], cq, r, eng, depth - 1):
                    set_waits(cand, [w])
                    return True
            return False

        for idx, i in enumerate(insts):
            if type(i).__name__ in SKIP:
                continue
            ws = waits_of(i)
            if len(ws) <= 1:
                continue
            eng = eng_of(i)
            ws.sort(key=lambda w: producer_idx(
                str(getattr(w, "ant_name", "")), w.wait_value))
            remaining = list(ws)
            progress = True
            while len(remaining) > 1 and progress:
                progress = False
                for w in list(remaining):
                    if len(remaining) <= 1:
                        break
                    q = producer_idx(str(getattr(w, "ant_name", "")),
                                     w.wait_value)
                    if place(w, q, idx, eng, 4):
                        remaining.remove(w)
                        progress = True
                        break
            assert len(remaining) <= 1, (
                f"{i.name} ({eng}): cannot reduce waits "
                f"{[(str(w.ant_name), w.wait_value) for w in remaining]} "
                f"producers "
                f"{[producer_idx(str(w.ant_name), w.wait_value) for w in remaining]} "
                f"at idx {idx}")
            set_waits(i, remaining)

        observed_sweep()


_CACHE = {}


def _get_module():
    if "nc" not in _CACHE:
        _CACHE["nc"] = build_module()
    return _CACHE["nc"]


def make_in_maps(x, context, Wq, Wk, Wv, Wo):
    import ml_dtypes
    bf = ml_dtypes.bfloat16
    x = np.asarray(x, np.float32).astype(bf)
    context = np.asarray(context, np.float32).astype(bf)
    Wq = np.asarray(Wq, np.float32).astype(bf)
    Wk = np.asarray(Wk, np.float32).astype(bf)
    Wv = np.asarray(Wv, np.float32).astype(bf)
    Wo = np.asarray(Wo, np.float32)
    cst = np.zeros((P, 644), np.float32)
    cst[:, 0:16] = 1.0
    cstf = np.zeros((33, P), np.float32)
    cstf[0, 0:64] = 1.0
    cstf[32, 64:128] = 1.0
    in_maps = []
    for c in range(N_CORES):
        b, g = divmod(c, 2)
        sl = slice(g * EL, (g + 1) * EL)
        in_maps.append({
            "xT": np.ascontiguousarray(x[b].T),
            "cT": np.ascontiguousarray(context[b].T),
            "wqT": np.ascontiguousarray(Wq[sl].T),
            "wkT": np.ascontiguousarray(Wk[sl].T),
            "wvT": np.ascontiguousarray(Wv[sl].T),
            "woT": np.ascontiguousarray(Wo[:, sl].T),
            "cst": cst,
            "cstf": cstf,
        })
    return in_maps


def gather_output(results, bo):
    bo = np.asarray(bo, np.float32)
    y = np.empty((B, NQ, D), np.float32)
    for b in range(B):
        y[b] = (results[2 * b]["yT"] + results[2 * b + 1]["yT"]).T + bo
    return y


def kernel(x, context, Wq, Wk, Wv, Wo, bo):
    nc = _get_module()
    in_maps = make_in_maps(x, context, Wq, Wk, Wv, Wo)
    res = run_bass_kernel_spmd(nc, in_maps, core_ids=list(range(N_CORES)))
    return gather_output(res.results, bo)

